# revision 9
# baseline (speedup 1.0000x reference)
"""Trainium2 Bass kernel for XCA-style cross-covariance attention.

Reference computation (per batch b):
    q = x1 @ Wq.T ; k = x2 @ Wk.T ; v = x2 @ Wv.T          # [N, C]
    per head h (d=64 channels): L2-normalize q,k along tokens,
    attn = softmax_e((qn^T kn) * temp)                      # [d, d]
    x_cross = attn @ v_h ; out = x_cross @ Wo.T + bo

Gram reformulation (token contractions become PSUM-accumulated Grams):
    S11 = x1^T x1, S21 = x2^T x1, S22 = x2^T x2             # [C, C]
    nq2[c] = sum_m Aq[m,c] * (S11 Aq)[m,c]   (Aq = Wq.T)    # ||q_col||^2
    nk2[c] likewise from S22, Ak
    T2raw = S21^T @ (Ak * temp[cols])        # temp folded into Ak early
    t2 = T2raw * (1/nk)[cols]                # column scaling commutes
    G_h = Aq[:,hb]^T t2[:,hb] ; attn_h = softmax(G_h * (1/nq)[rows])
    M[hb,:] = attn_h^T @ Wo.T[hb,:] ; W_eff = Wv.T @ M
    out = x2 @ W_eff + bo

Schedule (per core; data-parallel over batch B=8 -> 8 cores):
  phase 1 (DMA-paced ~47us): wq/wk/bo/temp first (~0.8us), then x1/x2
    interleaved 4-chunk batches on one queue (x1 through a 16-chunk ring,
    x2 fully staged); per chunk 6 Gram matmuls + 1 transpose of the x2
    chunk's first 128-col tile (f32r, 80ns); wv/wo load after the inputs.
  mid (~8us): serial C x C algebra, q/k chains overlapped across
    PE/DVE/Act, remaining early tile-1 transposes interleaved into PE
    wait gaps.
  phase 2 (store-paced ~24us): per chunk 2 matmuls into a 6-bank PSUM
    ring + 1 lookahead transpose; pair moves with bias add on DVE;
    4-chunk batched stores.
"""

import os
import sys

import numpy as np

_B, _N, _C, _H = 8, 8192, 256, 4
_P = 128  # SBUF partitions


def _ensure_paths():
    for p in ("/root/.axon_site/_ro/trn_rl_repo", "/opt/trn_rl_repo",
              "/root/.axon_site", "/root/.axon_site/_ro/pypackages"):
        if os.path.isdir(p) and p not in sys.path:
            sys.path.append(p)


def build_nc(n_tokens=_N):
    """Build the single-core Bass program (same program SPMD on 8 cores)."""
    _ensure_paths()
    import concourse.bass as bass
    import concourse.mybir as mybir
    import concourse.tile as tile
    from concourse import bacc
    from concourse.masks import make_identity

    f32 = mybir.dt.float32
    f32r = mybir.dt.float32r

    N, C, H = n_tokens, _C, _H
    P = _P
    NCH = N // P          # token chunks of 128
    CT = C // P           # channel tiles (2)
    RING = 16             # x1 ring depth in chunks
    OB = 4                # chunks per output store batch
    PB = 4                # phase-2 PSUM chunk ring depth
    LOOK = 8              # phase-2 transpose lookahead (chunks)

    # load batch sizes (4-chunk batches; soften the tail for earlier
    # last-chunk availability)
    sizes = []
    left = NCH
    while left > 8:
        sizes.append(4)
        left -= 4
    while left > 0:
        sizes.append(2)
        left -= 2

    nc = bacc.Bacc("TRN2", target_bir_lowering=False, debug=False)

    x1_d = nc.dram_tensor("x1", [N, C], f32, kind="ExternalInput").ap()
    x2_d = nc.dram_tensor("x2", [N, C], f32, kind="ExternalInput").ap()
    wq_d = nc.dram_tensor("Wq", [C, C], f32, kind="ExternalInput").ap()
    wk_d = nc.dram_tensor("Wk", [C, C], f32, kind="ExternalInput").ap()
    wv_d = nc.dram_tensor("Wv", [C, C], f32, kind="ExternalInput").ap()
    wo_d = nc.dram_tensor("Wo", [C, C], f32, kind="ExternalInput").ap()
    bo_d = nc.dram_tensor("bo", [C], f32, kind="ExternalInput").ap()
    tp_d = nc.dram_tensor("temperature", [H, 1, 1], f32, kind="ExternalInput").ap()
    out_d = nc.dram_tensor("out", [N, C], f32, kind="ExternalOutput").ap()

    with tile.TileContext(nc) as tc:
        with tc.tile_pool(name="consts", bufs=1) as consts, \
             tc.tile_pool(name="work", bufs=1, space="PSUM") as work:

            ident = consts.tile([P, P], f32, name="ident", tag="ident")
            make_identity(nc, ident)
            ones_f = consts.tile([P, P + 1], f32, name="ones_f", tag="ones_f")
            nc.vector.memset(ones_f, 1.0)
            ones_col = consts.tile([1, P], f32r, name="ones_col", tag="ones_col")
            nc.vector.tensor_copy(ones_col, ones_f[0:1, 0:P])
            ones_red = consts.tile([P, 1], f32r, name="ones_red", tag="ones_red")
            nc.vector.tensor_copy(ones_red, ones_f[:, 0:1])

            # ---- minimal weights first: Wq, Wk, bo, temperature ----
            wq_n = [consts.tile([P, C], f32, name=f"wq_n{t}", tag=f"wq_n{t}") for t in range(CT)]
            wk_n = [consts.tile([P, C], f32, name=f"wk_n{t}", tag=f"wk_n{t}") for t in range(CT)]
            for t in range(CT):
                nc.sync.dma_start(wq_n[t], wq_d[t * P:(t + 1) * P, :])
            for t in range(CT):
                nc.sync.dma_start(wk_n[t], wk_d[t * P:(t + 1) * P, :])
            bo_f = consts.tile([1, C], f32, name="bo_f", tag="bo_f")
            nc.sync.dma_start(bo_f, bo_d.partition_broadcast(1))
            tempsb = consts.tile([1, H], f32, name="tempsb", tag="tempsb")
            nc.sync.dma_start(tempsb, bass.AP(
                tensor=tp_d.tensor, offset=tp_d.offset, ap=[[0, 1], [1, H]]))

            # ---- input staging ----
            x2s = consts.tile([P, NCH, C], f32r, name="x2s", tag="x2s")
            x1s = consts.tile([P, RING, C], f32r, name="x1s", tag="x1s")

            pos = 0
            for s in sizes:
                src1 = bass.AP(tensor=x1_d.tensor,
                               offset=x1_d.offset + pos * P * C,
                               ap=[[C, P], [P * C, s], [1, C]]).bitcast(f32r)
                nc.sync.dma_start(x1s[:, pos % RING:pos % RING + s, :], src1)
                src2 = bass.AP(tensor=x2_d.tensor,
                               offset=x2_d.offset + pos * P * C,
                               ap=[[C, P], [P * C, s], [1, C]]).bitcast(f32r)
                nc.sync.dma_start(x2s[:, pos:pos + s, :], src2)
                pos += s

            # ---- remaining weights after the inputs on the queue ----
            wv_f = [consts.tile([P, C], f32, name=f"wv_f{t}", tag=f"wv_f{t}") for t in range(CT)]
            wo_n = [consts.tile([P, C], f32, name=f"wo_n{t}", tag=f"wo_n{t}") for t in range(CT)]
            for t in range(CT):
                nc.sync.dma_start(wv_f[t], wv_d[t * P:(t + 1) * P, :])
            for t in range(CT):
                nc.sync.dma_start(wo_n[t], wo_d[t * P:(t + 1) * P, :])

            # temperature -> flat per-channel row [1, C], then [P, 2C]
            # broadcast (rank-1) -> SBUF for folding into Ak
            tempflat = consts.tile([1, H, C // H], f32, name="tempflat", tag="tempflat")
            for h in range(H):
                nc.vector.tensor_scalar_mul(
                    tempflat[0:1, h, :], ones_f[0:1, 0:C // H],
                    tempsb[0:1, h:h + 1])
            tbrd_p = work.tile([P, 2 * C], f32, name="tbrd", tag="tp", bufs=2)
            tflat = tempflat.rearrange("a h j -> a (h j)")
            for t in range(CT):
                nc.tensor.matmul(tbrd_p[:, t * C:(t + 1) * C], ones_col,
                                 tflat.bitcast(f32r), start=(t == 0),
                                 stop=(t == CT - 1), skip_group_check=True)
            tempbrd = consts.tile([P, 2 * C], f32, name="tempbrd", tag="tempbrd")
            nc.vector.tensor_copy(tempbrd, tbrd_p)

            # bias broadcast rows [P, 2, C] for the phase-2 pair moves
            bob_p = work.tile([P, 2 * C], f32, name="bob", tag="tp", bufs=2)
            for t in range(2):
                nc.tensor.matmul(bob_p[:, t * C:(t + 1) * C], ones_col,
                                 bo_f.bitcast(f32r), start=(t == 0),
                                 stop=(t == 1), skip_group_check=True)
            bob2 = consts.tile([P, 2, C], f32, name="bob2", tag="bob2")
            nc.vector.tensor_copy(bob2.rearrange("p a c -> p (a c)"), bob_p)

            # transposed weights Aq=Wq.T, Ak=Wk.T (early; during DMA fill).
            # Dummy ident-transpose absorbs the identity (Pool) wait; tiny
            # absorber transposes attach each weight-DMA wait to a cheap PE
            # instruction so the real transposes need at most one wait.
            dummy = work.tile([P, P], f32, name="dummy", tag="tp", bufs=2)
            nc.tensor.transpose(dummy, ident, ident)
            for nat in (wq_n, wk_n):
                for tj in range(CT):
                    nc.tensor.transpose(dummy[0:32, :], nat[tj][:, 0:32], ident)

            aq = consts.tile([P, 2 * C], f32r, name="aq", tag="aq")
            ak_raw = consts.tile([P, 2 * C], f32r, name="ak_raw", tag="ak_raw")
            ak_tmp = consts.tile([P, 2 * C], f32r, name="ak_tmp", tag="ak_tmp")
            for ti in range(CT):
                tp = work.tile([P, C], f32, name="tp", tag="tp", bufs=2)
                for tj in range(CT):
                    nc.tensor.transpose(
                        tp[:, tj * P:(tj + 1) * P],
                        wq_n[tj][:, ti * P:(ti + 1) * P], ident)
                nc.vector.tensor_copy(aq[:, ti * C:(ti + 1) * C], tp)
            for ti in range(CT):
                tp = work.tile([P, C], f32, name="tp", tag="tp", bufs=2)
                for tj in range(CT):
                    nc.tensor.transpose(
                        tp[:, tj * P:(tj + 1) * P],
                        wk_n[tj][:, ti * P:(ti + 1) * P], ident)
                nc.vector.tensor_copy(ak_raw[:, ti * C:(ti + 1) * C], tp)
                nc.vector.tensor_mul(ak_tmp[:, ti * C:(ti + 1) * C], tp,
                                     tempbrd[:, ti * C:(ti + 1) * C])

            # ---- phase 1: Gram accumulation + tile-0 x2 transposes ----
            x2t = consts.tile([P, CT, N], f32r, name="x2t", tag="x2t")
            gram_cm = tc.tile_pool(name="gram", bufs=1, space="PSUM")
            gram = gram_cm.__enter__()
            s11p = gram.tile([P, 2 * C], f32, name="s11", tag="s11")
            s21p = gram.tile([P, 2 * C], f32, name="s21", tag="s21")
            s22p = gram.tile([P, 2 * C], f32, name="s22", tag="s22")

            ident_r = ident[:, :].bitcast(f32r)
            tp0 = None
            for i in range(NCH):
                x1c = x1s[:, i % RING, :]
                x2c = x2s[:, i, :]
                sp = (i == NCH - 1)
                for t in range(CT):
                    st = (i == 0) and (t == 0)
                    nc.tensor.matmul(
                        s11p[:, t * C:(t + 1) * C], x1c[:, t * P:(t + 1) * P],
                        x1c, start=st, stop=sp, skip_group_check=True)
                for t in range(CT):
                    st = (i == 0) and (t == 0)
                    nc.tensor.matmul(
                        s22p[:, t * C:(t + 1) * C], x2c[:, t * P:(t + 1) * P],
                        x2c, start=st, stop=sp, skip_group_check=True)
                for t in range(CT):
                    st = (i == 0) and (t == 0)
                    nc.tensor.matmul(
                        s21p[:, t * C:(t + 1) * C], x2c[:, t * P:(t + 1) * P],
                        x1c, start=st, stop=sp, skip_group_check=True)
                # tile-0 transpose of this chunk; pair-copy after odd chunks
                if i % 2 == 0:
                    tp0 = work.tile([P, 2, P], f32r, name="tp0", tag="tp0", bufs=2)
                nc.tensor.transpose(tp0[:, i % 2, :], x2c[:, 0:P], ident_r)
                if i % 2 == 1:
                    nc.vector.tensor_copy(
                        x2t[:, 0, (i - 1) * P:(i + 1) * P],
                        tp0.rearrange("p a q -> p (a q)"))

            # ---- post-gram weight processing (PE idle, DVE/Act free) ----
            ao = consts.tile([P, 2 * C], f32r, name="ao", tag="ao")
            for tj in range(CT):
                nc.tensor.transpose(dummy[0:32, :], wo_n[tj][:, 0:32], ident)
            for ti in range(CT):
                tp = work.tile([P, C], f32, name="tp", tag="tp", bufs=2)
                for tj in range(CT):
                    nc.tensor.transpose(
                        tp[:, tj * P:(tj + 1) * P],
                        wo_n[tj][:, ti * P:(ti + 1) * P], ident)
                nc.vector.tensor_copy(ao[:, ti * C:(ti + 1) * C], tp)

            # ---- mid phase ----
            # Gram copies: s22/s21 on DVE (k/T2 chains), s11 on Act.
            s22_sb = consts.tile([P, 2 * C], f32r, name="s22_sb", tag="s22_sb")
            nc.vector.tensor_copy(s22_sb, s22p)
            s21_sb = consts.tile([P, 2 * C], f32r, name="s21_sb", tag="s21_sb")
            nc.vector.tensor_copy(s21_sb, s21p)
            s11_sb = consts.tile([P, 2 * C], f32, name="s11_sb", tag="s11_sb")
            nc.scalar.copy(s11_sb, s11p)
            gram_cm.__exit__(None, None, None)

            midA_cm = tc.tile_pool(name="midA", bufs=1, space="PSUM")
            midA = midA_cm.__enter__()
            midB_cm = tc.tile_pool(name="midB", bufs=3, space="PSUM")
            midB = midB_cm.__enter__()

            # tile-1 transposes for chunks 0..LOOK-1, interleaved into PE
            # wait gaps below; Act pair-copies into x2t tile 1.
            tp1_tiles = []

            def t1_transpose(i):
                if i % 2 == 0:
                    tp1_tiles.append(work.tile([P, 2, P], f32r, name="tp1",
                                               tag="tp0", bufs=2))
                nc.tensor.transpose(tp1_tiles[-1][:, i % 2, :],
                                    x2s[:, i, P:2 * P], ident_r)

            # T2raw = S12 @ (Ak*temp)   [lhsT = s21 tiles]
            t2raw = midA.tile([P, 2 * C], f32, name="t2raw", tag="mA")
            for t in range(CT):
                for uu in range(CT):
                    nc.tensor.matmul(
                        t2raw[:, t * C:(t + 1) * C],
                        s21_sb[:, uu * C + t * P:uu * C + (t + 1) * P],
                        ak_tmp[:, uu * C:(uu + 1) * C],
                        start=(t == 0 and uu == 0), stop=(t == CT - 1 and uu == CT - 1),
                        skip_group_check=True)
            t1_transpose(0)
            t1_transpose(1)

            # u_k = S22 @ Ak ; u_q = S11 @ Aq  (norm chains)
            u_k = midB.tile([P, 2 * C], f32, name="u_k", tag="mB")
            for t in range(CT):
                for uu in range(CT):
                    nc.tensor.matmul(
                        u_k[:, t * C:(t + 1) * C],
                        s22_sb[:, uu * C + t * P:uu * C + (t + 1) * P],
                        ak_raw[:, uu * C:(uu + 1) * C],
                        start=(t == 0 and uu == 0), stop=(t == CT - 1 and uu == CT - 1),
                        skip_group_check=True)
            t1_transpose(2)
            t1_transpose(3)
            u_q = midB.tile([P, 2 * C], f32, name="u_q", tag="mB")
            for t in range(CT):
                for uu in range(CT):
                    nc.tensor.matmul(
                        u_q[:, t * C:(t + 1) * C],
                        s11_sb[:, uu * C + t * P:uu * C + (t + 1) * P].bitcast(f32r),
                        aq[:, uu * C:(uu + 1) * C],
                        start=(t == 0 and uu == 0), stop=(t == CT - 1 and uu == CT - 1),
                        skip_group_check=True)

            vv_k = consts.tile([P, 2 * C], f32r, name="vv_k", tag="vv_k")
            nc.vector.tensor_mul(vv_k, ak_raw, u_k)
            vv_q = consts.tile([P, 2 * C], f32r, name="vv_q", tag="vv_q")
            nc.vector.tensor_mul(vv_q, aq, u_q)

            # nf bank: [1, 2C]: k at cols 0:C, q at cols C:2C
            nf = midB.tile([1, 2 * C], f32, name="nf", tag="mB")
            for t in range(CT):
                nc.tensor.matmul(nf[0:1, 0:C], ones_red,
                                 vv_k[:, t * C:(t + 1) * C],
                                 start=(t == 0), stop=(t == CT - 1),
                                 skip_group_check=True)
            for t in range(CT):
                nc.tensor.matmul(nf[0:1, C:2 * C], ones_red,
                                 vv_q[:, t * C:(t + 1) * C],
                                 start=False, stop=(t == CT - 1),
                                 skip_group_check=True)
            t1_transpose(4)
            t1_transpose(5)

            nk_len = consts.tile([1, C], f32, name="nk_len", tag="nk_len")
            nc.scalar.sqrt(nk_len, nf[0:1, 0:C])
            nq_len = consts.tile([1, C], f32, name="nq_len", tag="nq_len")
            nc.scalar.sqrt(nq_len, nf[0:1, C:2 * C])
            nk_inv = consts.tile([1, C], f32, name="nk_inv", tag="nk_inv")
            nc.vector.reciprocal(nk_inv, nk_len)
            nq_inv = consts.tile([1, C], f32, name="nq_inv", tag="nq_inv")
            nc.vector.reciprocal(nq_inv, nq_len)

            # bnk2 = broadcast(1/nk) twice along free dim  [P, 2C]
            bnk2 = midB.tile([P, 2 * C], f32, name="bnk2", tag="mB")
            for t in range(CT):
                nc.tensor.matmul(bnk2[:, t * C:(t + 1) * C], ones_col,
                                 nk_inv.bitcast(f32r), start=(t == 0),
                                 stop=(t == CT - 1), skip_group_check=True)
            t1_transpose(6)
            t1_transpose(7)
            bnk_sb = consts.tile([P, 2 * C], f32, name="bnk_sb", tag="bnk_sb")
            nc.scalar.copy(bnk_sb, bnk2)
            # first two tile-1 pair-copies (Act is idle until the Exps)
            for pr in (0, 1):
                nc.scalar.copy(
                    x2t[:, 1, 2 * pr * P:2 * (pr + 1) * P].bitcast(f32),
                    tp1_tiles[pr].rearrange("p a q -> p (a q)").bitcast(f32))

            # t2 = T2raw * (1/nk)[cols]  (by uu halves for G pipelining)
            t2s = consts.tile([P, 2 * C], f32r, name="t2s", tag="t2s")
            nc.vector.tensor_mul(t2s[:, 0:C], t2raw[:, 0:C], bnk_sb[:, 0:C])

            # G pairs + rowscale into one bank [P, 132]
            gbank = work.tile([P, 132], f32, name="gbank", tag="tp", bufs=2)
            for uu in range(CT):
                if uu == 1:
                    nc.vector.tensor_mul(t2s[:, C:2 * C], t2raw[:, C:2 * C],
                                         bnk_sb[:, C:2 * C])
                for t in range(2):      # head pair (2t, 2t+1)
                    for par in range(2):
                        h = 2 * t + par
                        hb = slice(h * 64, (h + 1) * 64)
                        nc.tensor.matmul(
                            gbank[par * 64:(par + 1) * 64, t * 64:(t + 1) * 64],
                            aq[:, uu * C + h * 64:uu * C + (h + 1) * 64],
                            t2s[:, uu * C + h * 64:uu * C + (h + 1) * 64],
                            start=(uu == 0 and t == 0 and par == 0),
                            stop=(uu == CT - 1), skip_group_check=True)
                if uu == 0:
                    # rowscale columns (after the bank's start-zeroing)
                    for t in range(2):
                        nc.tensor.matmul(
                            gbank[:, 128 + 2 * t:129 + 2 * t],
                            nq_inv[0:1, t * P:(t + 1) * P].bitcast(f32r),
                            ones_col[0:1, 0:1],
                            start=False, stop=True, skip_group_check=True)
            rowscale = consts.tile([P, 4], f32, name="rowscale", tag="rowscale")
            nc.vector.tensor_copy(rowscale, gbank[:, 128:132])

            # softmax per pair + M + W_eff
            mm_bank = midB.tile([P, 2 * C], f32, name="mmb", tag="mB")
            at2 = []
            for t in range(2):
                ex = consts.tile([P, 64], f32, name=f"ex{t}", tag=f"ex{t}")
                sume = consts.tile([P, 1], f32, name=f"se{t}", tag=f"se{t}")
                nc.scalar.activation(
                    ex, gbank[:, t * 64:(t + 1) * 64],
                    mybir.ActivationFunctionType.Exp,
                    scale=rowscale[:, 2 * t:2 * t + 1], accum_out=sume)
                sinv = consts.tile([P, 1], f32, name=f"si{t}", tag=f"si{t}")
                nc.vector.reciprocal(sinv, sume)
                at_t = consts.tile([P, 64], f32r, name=f"at{t}", tag=f"at{t}")
                nc.vector.tensor_scalar_mul(at_t, ex, sinv)
                at2.append(at_t)
                for par in range(2):
                    sl = slice(par * 64, (par + 1) * 64)
                    nc.tensor.matmul(
                        mm_bank[sl, t * C:(t + 1) * C], at2[t][sl, :],
                        ao[sl.start:sl.stop, t * C:(t + 1) * C],
                        start=(t == 0 and par == 0), stop=True,
                        skip_group_check=True)

            mm_sb = consts.tile([P, 2 * C], f32r, name="mm_sb", tag="mm_sb")
            weffb = midB.tile([P, 2 * C], f32, name="weffb", tag="mB")
            weff_sb = consts.tile([P, 2 * C], f32r, name="weff_sb", tag="weff_sb")
            for uu in range(CT):
                nc.vector.tensor_copy(mm_sb[:, uu * C:(uu + 1) * C],
                                      mm_bank[:, uu * C:(uu + 1) * C])
                for t in range(CT):
                    nc.tensor.matmul(
                        weffb[:, t * C:(t + 1) * C],
                        wv_f[uu][:, t * P:(t + 1) * P].bitcast(f32r),
                        mm_sb[:, uu * C:(uu + 1) * C],
                        start=(uu == 0), stop=(uu == CT - 1),
                        skip_group_check=True)
            for t in range(CT):
                nc.vector.tensor_copy(weff_sb[:, t * C:(t + 1) * C],
                                      weffb[:, t * C:(t + 1) * C])
            # remaining tile-1 pair-copies on Act
            for pr in (2, 3):
                nc.scalar.copy(
                    x2t[:, 1, 2 * pr * P:2 * (pr + 1) * P].bitcast(f32),
                    tp1_tiles[pr].rearrange("p a q -> p (a q)").bitcast(f32))

            midB_cm.__exit__(None, None, None)
            midA_cm.__exit__(None, None, None)

            # ---- phase 2: out = x2 @ W_eff + bo ----
            p2_cm = tc.tile_pool(name="p2", bufs=1, space="PSUM")
            p2pool = p2_cm.__enter__()
            p2 = p2pool.tile([P, PB, 2 * C], f32, name="p2", tag="p2")
            ostr = consts.tile([P, 2, OB, C], f32, name="ostr", tag="ostr")

            tpl = None
            for i in range(NCH):
                j = i + LOOK
                if j < NCH:
                    if j % 2 == 0:
                        tpl = work.tile([P, 2, P], f32r, name="tpl",
                                        tag="tp0", bufs=2)
                    nc.tensor.transpose(tpl[:, j % 2, :],
                                        x2s[:, j, P:2 * P], ident_r)
                b = i % PB
                for t in range(CT):
                    nc.tensor.matmul(
                        p2[:, b, 0:C],
                        x2t[:, t, i * P:(i + 1) * P],
                        weff_sb[:, t * C:(t + 1) * C],
                        start=(t == 0), stop=(t == CT - 1),
                        skip_group_check=True)
                if j < NCH and j % 2 == 1:
                    nc.scalar.copy(
                        x2t[:, 1, (j - 1) * P:(j + 1) * P].bitcast(f32),
                        tpl.rearrange("p a q -> p (a q)").bitcast(f32))
                if i % 2 == 1:
                    half = (i // OB) % 2
                    nc.vector.tensor_add(
                        ostr[:, half, (i % OB) - 1:(i % OB) + 1, :],
                        p2[:, b - 1:b + 1, 0:C], bob2)
                if i % OB == OB - 1:
                    b0 = i - OB + 1
                    half = (i // OB) % 2
                    dst = bass.AP(
                        tensor=out_d.tensor,
                        offset=out_d.offset + b0 * P * C,
                        ap=[[C, P], [P * C, OB], [1, C]])
                    nc.sync.dma_start(dst, ostr[:, half, :, :])
            p2_cm.__exit__(None, None, None)

    nc.compile()
    return nc


_NC_CACHE = {}


def _get_nc(n_tokens=_N):
    if n_tokens not in _NC_CACHE:
        _NC_CACHE[n_tokens] = build_nc(n_tokens)
    return _NC_CACHE[n_tokens]


def kernel(x1, x2, Wq, Wk, Wv, Wo, bo, temperature):
    _ensure_paths()
    from concourse.bass_utils import run_bass_kernel_spmd

    B = x1.shape[0]
    nc = _get_nc(x1.shape[1])
    in_maps = []
    for b in range(B):
        in_maps.append({
            "x1": np.ascontiguousarray(x1[b], dtype=np.float32),
            "x2": np.ascontiguousarray(x2[b], dtype=np.float32),
            "Wq": np.asarray(Wq, dtype=np.float32),
            "Wk": np.asarray(Wk, dtype=np.float32),
            "Wv": np.asarray(Wv, dtype=np.float32),
            "Wo": np.asarray(Wo, dtype=np.float32),
            "bo": np.asarray(bo, dtype=np.float32),
            "temperature": np.asarray(temperature, dtype=np.float32),
        })
    res = run_bass_kernel_spmd(nc, in_maps, core_ids=list(range(B)))
    return np.stack([res.results[b]["out"] for b in range(B)]).astype(np.float32)


# revision 17
# speedup vs baseline: 1.2113x; 1.2113x over previous
"""Trainium2 Bass kernel for XCA-style cross-covariance attention.

Reference computation (per batch b):
    q = x1 @ Wq.T ; k = x2 @ Wk.T ; v = x2 @ Wv.T          # [N, C]
    per head h (d=64 channels): L2-normalize q,k along tokens,
    attn = softmax_e((qn^T kn) * temp)                      # [d, d]
    x_cross = attn @ v_h ; out = x_cross @ Wo.T + bo

Gram reformulation (token contractions become PSUM-accumulated Grams):
    S11 = x1^T x1, S21 = x2^T x1, S22 = x2^T x2             # [C, C]
    nq2[c] = sum_m Aq[m,c] * (S11 Aq)[m,c]   (Aq = Wq.T)    # ||q_col||^2
    nk2[c] likewise from S22, Ak
    T2raw = S21^T @ (Ak * temp[cols])        # temp folded into Ak early
    t2 = T2raw * (1/nk)[cols]                # column scaling commutes
    G_h = Aq[:,hb]^T t2[:,hb] ; attn_h = softmax(G_h * (1/nq)[rows])
    M[hb,:] = attn_h^T @ Wo.T[hb,:] ; W_eff = Wv.T @ M
    out = x2 @ W_eff + bo

Schedule (per core; data-parallel over batch B=8 -> 8 cores):
  phase 1 (DMA-paced ~47us): wq/wk/bo/temp first (~0.8us), then x1/x2
    interleaved 4-chunk batches on one queue (x1 through a 16-chunk ring,
    x2 fully staged); per chunk 6 Gram matmuls + 1 transpose of the x2
    chunk's first 128-col tile (f32r, 80ns); wv/wo load after the inputs.
  mid (~8us): serial C x C algebra, q/k chains overlapped across
    PE/DVE/Act, remaining early tile-1 transposes interleaved into PE
    wait gaps.
  phase 2 (store-paced ~24us): per chunk 2 matmuls into a 6-bank PSUM
    ring + 1 lookahead transpose; pair moves with bias add on DVE;
    4-chunk batched stores.
"""

import os
import sys

import numpy as np

_B, _N, _C, _H = 8, 8192, 256, 4
_P = 128  # SBUF partitions


def _ensure_paths():
    for p in ("/root/.axon_site/_ro/trn_rl_repo", "/opt/trn_rl_repo",
              "/root/.axon_site", "/root/.axon_site/_ro/pypackages"):
        if os.path.isdir(p) and p not in sys.path:
            sys.path.append(p)


def build_nc(n_tokens=_N):
    """Build the single-core Bass program (same program SPMD on 8 cores)."""
    _ensure_paths()
    import concourse.bass as bass
    import concourse.mybir as mybir
    import concourse.tile as tile
    from concourse import bacc
    from concourse.masks import make_identity

    f32 = mybir.dt.float32
    f32r = mybir.dt.float32r

    N, C, H = n_tokens, _C, _H
    P = _P
    NCH = N // P          # token chunks of 128
    CT = C // P           # channel tiles (2)
    RING = 16             # x1 ring depth in chunks
    OB = 4                # chunks per output store batch
    PB = 4                # phase-2 PSUM chunk ring depth
    LOOK = 8              # phase-2 transpose lookahead (chunks)

    # load batch sizes (4-chunk batches; soften the tail for earlier
    # last-chunk availability)
    sizes = []
    left = NCH
    while left > 8:
        sizes.append(4)
        left -= 4
    while left > 0:
        sizes.append(2)
        left -= 2

    nc = bacc.Bacc("TRN2", target_bir_lowering=False, debug=False)

    x1_d = nc.dram_tensor("x1", [N, C], f32, kind="ExternalInput").ap()
    x2_d = nc.dram_tensor("x2", [N, C], f32, kind="ExternalInput").ap()
    wq_d = nc.dram_tensor("Wq", [C, C], f32, kind="ExternalInput").ap()
    wk_d = nc.dram_tensor("Wk", [C, C], f32, kind="ExternalInput").ap()
    wv_d = nc.dram_tensor("Wv", [C, C], f32, kind="ExternalInput").ap()
    wo_d = nc.dram_tensor("Wo", [C, C], f32, kind="ExternalInput").ap()
    bo_d = nc.dram_tensor("bo", [C], f32, kind="ExternalInput").ap()
    tp_d = nc.dram_tensor("temperature", [H, 1, 1], f32, kind="ExternalInput").ap()
    out_d = nc.dram_tensor("out", [N, C], f32, kind="ExternalOutput").ap()

    with tile.TileContext(nc) as tc:
        with tc.tile_pool(name="consts", bufs=1) as consts, \
             tc.tile_pool(name="work", bufs=1, space="PSUM") as work:

            ident = consts.tile([P, P], f32, name="ident", tag="ident")
            make_identity(nc, ident)
            ones_f = consts.tile([P, P + 1], f32, name="ones_f", tag="ones_f")
            nc.vector.memset(ones_f, 1.0)
            ones_col = consts.tile([1, P], f32r, name="ones_col", tag="ones_col")
            nc.vector.tensor_copy(ones_col, ones_f[0:1, 0:P])
            ones_red = consts.tile([P, 1], f32r, name="ones_red", tag="ones_red")
            nc.vector.tensor_copy(ones_red, ones_f[:, 0:1])

            # ---- minimal weights first: Wq, Wk, bo, temperature ----
            wq_n = [consts.tile([P, C], f32, name=f"wq_n{t}", tag=f"wq_n{t}") for t in range(CT)]
            wk_n = [consts.tile([P, C], f32, name=f"wk_n{t}", tag=f"wk_n{t}") for t in range(CT)]
            for t in range(CT):
                nc.sync.dma_start(wq_n[t], wq_d[t * P:(t + 1) * P, :])
            for t in range(CT):
                nc.sync.dma_start(wk_n[t], wk_d[t * P:(t + 1) * P, :])
            bo_f = consts.tile([1, C], f32, name="bo_f", tag="bo_f")
            nc.sync.dma_start(bo_f, bo_d.partition_broadcast(1))
            tempsb = consts.tile([1, H], f32, name="tempsb", tag="tempsb")
            nc.sync.dma_start(tempsb, bass.AP(
                tensor=tp_d.tensor, offset=tp_d.offset, ap=[[0, 1], [1, H]]))

            # ---- input staging ----
            # x2 fully resident (no reuse -> no WAR); x1 streams through a
            # 4-batch ring of per-batch pool tiles (WAR at slot granularity).
            x2s = consts.tile([P, NCH, C], f32r, name="x2s", tag="x2s")
            x1pool_cm = tc.tile_pool(name="x1p", bufs=4)
            x1pool = x1pool_cm.__enter__()

            bpos = []
            pos = 0
            for s in sizes:
                bpos.append(pos)
                pos += s
            NB = len(sizes)
            chunk_batch = {}
            for k in range(NB):
                for i in range(bpos[k], bpos[k] + sizes[k]):
                    chunk_batch[i] = k
            # emit batch k's loads at end of chunk last(k-4) (ring WAR pred)
            emit_at = {}
            for k in range(4, NB):
                emit_at.setdefault(bpos[k - 4] + sizes[k - 4] - 1, []).append(k)

            x1tiles = {}

            def load_pair(k):
                s = sizes[k]
                p0 = bpos[k]
                xt = x1pool.tile([P, 4, C], f32r, name=f"x1b{k}", tag="x1b",
                                 bufs=4)
                x1tiles[k] = xt
                src1 = bass.AP(tensor=x1_d.tensor,
                               offset=x1_d.offset + p0 * P * C,
                               ap=[[C, P], [P * C, s], [1, C]]).bitcast(f32r)
                nc.sync.dma_start(xt[:, 0:s, :], src1)
                src2 = bass.AP(tensor=x2_d.tensor,
                               offset=x2_d.offset + p0 * P * C,
                               ap=[[C, P], [P * C, s], [1, C]]).bitcast(f32r)
                nc.sync.dma_start(x2s[:, p0:p0 + s, :], src2)

            for k in range(4):
                load_pair(k)

            # wv/wo load after the phase-1 inputs (emitted later; they are
            # only needed by the mid phase)
            wv_f = [consts.tile([P, C], f32, name=f"wv_f{t}", tag=f"wv_f{t}") for t in range(CT)]
            wo_n = [consts.tile([P, C], f32, name=f"wo_n{t}", tag=f"wo_n{t}") for t in range(CT)]

            # temperature -> flat per-channel row [1, C], then [P, 2C]
            # broadcast (rank-1) -> SBUF for folding into Ak
            tempflat = consts.tile([1, H, C // H], f32, name="tempflat", tag="tempflat")
            for h in range(H):
                nc.vector.tensor_scalar_mul(
                    tempflat[0:1, h, :], ones_f[0:1, 0:C // H],
                    tempsb[0:1, h:h + 1])
            tbrd_p = work.tile([P, 2 * C], f32, name="tbrd", tag="tp", bufs=2)
            tflat = tempflat.rearrange("a h j -> a (h j)")
            for t in range(CT):
                nc.tensor.matmul(tbrd_p[:, t * C:(t + 1) * C], ones_col,
                                 tflat.bitcast(f32r), start=(t == 0),
                                 stop=(t == CT - 1), skip_group_check=True)
            tempbrd = consts.tile([P, 2 * C], f32, name="tempbrd", tag="tempbrd")
            nc.vector.tensor_copy(tempbrd, tbrd_p)

            # bias broadcast rows [P, 2, C] for the phase-2 pair moves
            bob_p = work.tile([P, 2 * C], f32, name="bob", tag="tp", bufs=2)
            for t in range(2):
                nc.tensor.matmul(bob_p[:, t * C:(t + 1) * C], ones_col,
                                 bo_f.bitcast(f32r), start=(t == 0),
                                 stop=(t == 1), skip_group_check=True)
            bob2 = consts.tile([P, 2, C], f32, name="bob2", tag="bob2")
            nc.vector.tensor_copy(bob2.rearrange("p a c -> p (a c)"), bob_p)

            # transposed weights Aq=Wq.T, Ak=Wk.T (early; during DMA fill).
            # Dummy ident-transpose absorbs the identity (Pool) wait; tiny
            # absorber transposes attach each weight-DMA wait to a cheap PE
            # instruction so the real transposes need at most one wait.
            dummy = work.tile([P, P], f32, name="dummy", tag="tp", bufs=2)
            nc.tensor.transpose(dummy, ident, ident)
            for nat in (wq_n, wk_n):
                for tj in range(CT):
                    nc.tensor.transpose(dummy[0:32, :], nat[tj][:, 0:32], ident)

            aq = consts.tile([P, 2 * C], f32r, name="aq", tag="aq")
            ak_raw = consts.tile([P, 2 * C], f32r, name="ak_raw", tag="ak_raw")
            ak_tmp = consts.tile([P, 2 * C], f32r, name="ak_tmp", tag="ak_tmp")
            for ti in range(CT):
                tp = work.tile([P, C], f32, name="tp", tag="tp", bufs=2)
                for tj in range(CT):
                    nc.tensor.transpose(
                        tp[:, tj * P:(tj + 1) * P],
                        wq_n[tj][:, ti * P:(ti + 1) * P], ident)
                nc.vector.tensor_copy(aq[:, ti * C:(ti + 1) * C], tp)
            for ti in range(CT):
                tp = work.tile([P, C], f32, name="tp", tag="tp", bufs=2)
                for tj in range(CT):
                    nc.tensor.transpose(
                        tp[:, tj * P:(tj + 1) * P],
                        wk_n[tj][:, ti * P:(ti + 1) * P], ident)
                nc.vector.tensor_copy(ak_raw[:, ti * C:(ti + 1) * C], tp)
                nc.vector.tensor_mul(ak_tmp[:, ti * C:(ti + 1) * C], tp,
                                     tempbrd[:, ti * C:(ti + 1) * C])

            # ---- phase 1: Gram accumulation + tile-0 x2 transposes ----
            x2t = consts.tile([P, CT, N], f32r, name="x2t", tag="x2t")
            gram_cm = tc.tile_pool(name="gram", bufs=1, space="PSUM")
            gram = gram_cm.__enter__()
            s11p = gram.tile([P, 2 * C], f32, name="s11", tag="s11")
            s21p = gram.tile([P, 2 * C], f32, name="s21", tag="s21")
            s22p = gram.tile([P, 2 * C], f32, name="s22", tag="s22")

            ident_r = ident[:, :].bitcast(f32r)
            tp0 = None
            for i in range(NCH):
                kb = chunk_batch[i]
                x1c = x1tiles[kb][:, i - bpos[kb], :]
                x2c = x2s[:, i, :]
                sp = (i == NCH - 1)
                for t in range(CT):
                    st = (i == 0) and (t == 0)
                    nc.tensor.matmul(
                        s11p[:, t * C:(t + 1) * C], x1c[:, t * P:(t + 1) * P],
                        x1c, start=st, stop=sp, skip_group_check=True)
                for t in range(CT):
                    st = (i == 0) and (t == 0)
                    nc.tensor.matmul(
                        s22p[:, t * C:(t + 1) * C], x2c[:, t * P:(t + 1) * P],
                        x2c, start=st, stop=sp, skip_group_check=True)
                for t in range(CT):
                    st = (i == 0) and (t == 0)
                    nc.tensor.matmul(
                        s21p[:, t * C:(t + 1) * C], x2c[:, t * P:(t + 1) * P],
                        x1c, start=st, stop=sp, skip_group_check=True)
                # tile-0 transpose of this chunk; pair-copy after odd chunks
                if i % 2 == 0:
                    tp0 = work.tile([P, 2, P], f32r, name="tp0", tag="tp0", bufs=2)
                nc.tensor.transpose(tp0[:, i % 2, :], x2c[:, 0:P], ident_r)
                if i % 2 == 1:
                    nc.vector.tensor_copy(
                        x2t[:, 0, (i - 1) * P:(i + 1) * P],
                        tp0.rearrange("p a q -> p (a q)"))
                for k in emit_at.get(i, []):
                    load_pair(k)
            for t in range(CT):
                nc.sync.dma_start(wv_f[t], wv_d[t * P:(t + 1) * P, :])
            for t in range(CT):
                nc.sync.dma_start(wo_n[t], wo_d[t * P:(t + 1) * P, :])
            x1pool_cm.__exit__(None, None, None)

            # ---- post-gram weight processing (PE idle, DVE/Act free) ----
            ao = consts.tile([P, 2 * C], f32r, name="ao", tag="ao")
            for tj in range(CT):
                nc.tensor.transpose(dummy[0:32, :], wo_n[tj][:, 0:32], ident)
            for ti in range(CT):
                tp = work.tile([P, C], f32, name="tp", tag="tp", bufs=2)
                for tj in range(CT):
                    nc.tensor.transpose(
                        tp[:, tj * P:(tj + 1) * P],
                        wo_n[tj][:, ti * P:(ti + 1) * P], ident)
                nc.vector.tensor_copy(ao[:, ti * C:(ti + 1) * C], tp)

            # ---- mid phase ----
            # Gram copies: s22/s21 on DVE (k/T2 chains), s11 on Act.
            s22_sb = consts.tile([P, 2 * C], f32r, name="s22_sb", tag="s22_sb")
            nc.vector.tensor_copy(s22_sb, s22p)
            s21_sb = consts.tile([P, 2 * C], f32r, name="s21_sb", tag="s21_sb")
            nc.vector.tensor_copy(s21_sb, s21p)
            s11_sb = consts.tile([P, 2 * C], f32, name="s11_sb", tag="s11_sb")
            nc.scalar.copy(s11_sb, s11p)
            gram_cm.__exit__(None, None, None)

            midA_cm = tc.tile_pool(name="midA", bufs=1, space="PSUM")
            midA = midA_cm.__enter__()
            midB_cm = tc.tile_pool(name="midB", bufs=3, space="PSUM")
            midB = midB_cm.__enter__()

            # tile-1 transposes for chunks 0..LOOK-1, interleaved into PE
            # wait gaps below; Act pair-copies into x2t tile 1.
            tp1_tiles = []

            def t1_transpose(i):
                if i % 2 == 0:
                    tp1_tiles.append(work.tile([P, 2, P], f32r, name="tp1",
                                               tag="tp0", bufs=2))
                nc.tensor.transpose(tp1_tiles[-1][:, i % 2, :],
                                    x2s[:, i, P:2 * P], ident_r)

            # T2raw = S12 @ (Ak*temp)   [lhsT = s21 tiles]
            t2raw = midA.tile([P, 2 * C], f32, name="t2raw", tag="mA")
            for t in range(CT):
                for uu in range(CT):
                    nc.tensor.matmul(
                        t2raw[:, t * C:(t + 1) * C],
                        s21_sb[:, uu * C + t * P:uu * C + (t + 1) * P],
                        ak_tmp[:, uu * C:(uu + 1) * C],
                        start=(t == 0 and uu == 0), stop=(t == CT - 1 and uu == CT - 1),
                        skip_group_check=True)
            t1_transpose(0)
            t1_transpose(1)
            nc.scalar.copy(
                x2t[:, 1, 0:2 * P].bitcast(f32),
                tp1_tiles[0].rearrange("p a q -> p (a q)").bitcast(f32))

            # u_k = S22 @ Ak ; u_q = S11 @ Aq  (norm chains)
            u_k = midB.tile([P, 2 * C], f32, name="u_k", tag="mB")
            for t in range(CT):
                for uu in range(CT):
                    nc.tensor.matmul(
                        u_k[:, t * C:(t + 1) * C],
                        s22_sb[:, uu * C + t * P:uu * C + (t + 1) * P],
                        ak_raw[:, uu * C:(uu + 1) * C],
                        start=(t == 0 and uu == 0), stop=(t == CT - 1 and uu == CT - 1),
                        skip_group_check=True)
            t1_transpose(2)
            t1_transpose(3)
            nc.scalar.copy(
                x2t[:, 1, 2 * P:4 * P].bitcast(f32),
                tp1_tiles[1].rearrange("p a q -> p (a q)").bitcast(f32))
            u_q = midB.tile([P, 2 * C], f32, name="u_q", tag="mB")
            for t in range(CT):
                for uu in range(CT):
                    nc.tensor.matmul(
                        u_q[:, t * C:(t + 1) * C],
                        s11_sb[:, uu * C + t * P:uu * C + (t + 1) * P].bitcast(f32r),
                        aq[:, uu * C:(uu + 1) * C],
                        start=(t == 0 and uu == 0), stop=(t == CT - 1 and uu == CT - 1),
                        skip_group_check=True)

            vv_k = consts.tile([P, 2 * C], f32r, name="vv_k", tag="vv_k")
            nc.vector.tensor_mul(vv_k, ak_raw, u_k)
            vv_q = consts.tile([P, 2 * C], f32r, name="vv_q", tag="vv_q")
            nc.vector.tensor_mul(vv_q, aq, u_q)

            # nf bank: [1, 2C]: k at cols 0:C, q at cols C:2C
            nf = midB.tile([1, 2 * C], f32, name="nf", tag="mB")
            for t in range(CT):
                nc.tensor.matmul(nf[0:1, 0:C], ones_red,
                                 vv_k[:, t * C:(t + 1) * C],
                                 start=(t == 0), stop=(t == CT - 1),
                                 skip_group_check=True)
            for t in range(CT):
                nc.tensor.matmul(nf[0:1, C:2 * C], ones_red,
                                 vv_q[:, t * C:(t + 1) * C],
                                 start=False, stop=(t == CT - 1),
                                 skip_group_check=True)
            t1_transpose(4)
            t1_transpose(5)

            nk_len = consts.tile([1, C], f32, name="nk_len", tag="nk_len")
            nc.scalar.sqrt(nk_len, nf[0:1, 0:C])
            nq_len = consts.tile([1, C], f32, name="nq_len", tag="nq_len")
            nc.scalar.sqrt(nq_len, nf[0:1, C:2 * C])
            nk_inv = consts.tile([1, C], f32, name="nk_inv", tag="nk_inv")
            nc.vector.reciprocal(nk_inv, nk_len)
            nq_inv = consts.tile([1, C], f32, name="nq_inv", tag="nq_inv")
            nc.vector.reciprocal(nq_inv, nq_len)

            # bnk2 = broadcast(1/nk) twice along free dim  [P, 2C]
            bnk2 = midB.tile([P, 2 * C], f32, name="bnk2", tag="mB")
            for t in range(CT):
                nc.tensor.matmul(bnk2[:, t * C:(t + 1) * C], ones_col,
                                 nk_inv.bitcast(f32r), start=(t == 0),
                                 stop=(t == CT - 1), skip_group_check=True)
            t1_transpose(6)
            t1_transpose(7)
            bnk_sb = consts.tile([P, 2 * C], f32, name="bnk_sb", tag="bnk_sb")
            nc.scalar.copy(bnk_sb, bnk2)
            nc.scalar.copy(
                x2t[:, 1, 4 * P:6 * P].bitcast(f32),
                tp1_tiles[2].rearrange("p a q -> p (a q)").bitcast(f32))

            # t2 = T2raw * (1/nk)[cols]  (by uu halves for G pipelining)
            t2s = consts.tile([P, 2 * C], f32r, name="t2s", tag="t2s")
            nc.vector.tensor_mul(t2s[:, 0:C], t2raw[:, 0:C], bnk_sb[:, 0:C])

            # G pairs + rowscale into one bank [P, 132]
            gbank = work.tile([P, 132], f32, name="gbank", tag="tp", bufs=2)
            for uu in range(CT):
                if uu == 1:
                    nc.vector.tensor_mul(t2s[:, C:2 * C], t2raw[:, C:2 * C],
                                         bnk_sb[:, C:2 * C])
                for t in range(2):      # head pair (2t, 2t+1)
                    for par in range(2):
                        h = 2 * t + par
                        hb = slice(h * 64, (h + 1) * 64)
                        nc.tensor.matmul(
                            gbank[par * 64:(par + 1) * 64, t * 64:(t + 1) * 64],
                            aq[:, uu * C + h * 64:uu * C + (h + 1) * 64],
                            t2s[:, uu * C + h * 64:uu * C + (h + 1) * 64],
                            start=(uu == 0 and t == 0 and par == 0),
                            stop=(uu == CT - 1), skip_group_check=True)
                if uu == 0:
                    # rowscale columns (after the bank's start-zeroing)
                    for t in range(2):
                        nc.tensor.matmul(
                            gbank[:, 128 + 2 * t:129 + 2 * t],
                            nq_inv[0:1, t * P:(t + 1) * P].bitcast(f32r),
                            ones_col[0:1, 0:1],
                            start=False, stop=True, skip_group_check=True)
            rowscale = consts.tile([P, 4], f32, name="rowscale", tag="rowscale")
            nc.vector.tensor_copy(rowscale, gbank[:, 128:132])

            # softmax per pair + M + W_eff
            mm_bank = midB.tile([P, 2 * C], f32, name="mmb", tag="mB")
            at2 = []
            for t in range(2):
                ex = consts.tile([P, 64], f32, name=f"ex{t}", tag=f"ex{t}")
                sume = consts.tile([P, 1], f32, name=f"se{t}", tag=f"se{t}")
                nc.scalar.activation(
                    ex, gbank[:, t * 64:(t + 1) * 64],
                    mybir.ActivationFunctionType.Exp,
                    scale=rowscale[:, 2 * t:2 * t + 1], accum_out=sume)
                sinv = consts.tile([P, 1], f32, name=f"si{t}", tag=f"si{t}")
                nc.vector.reciprocal(sinv, sume)
                at_t = consts.tile([P, 64], f32r, name=f"at{t}", tag=f"at{t}")
                nc.vector.tensor_scalar_mul(at_t, ex, sinv)
                at2.append(at_t)
                for par in range(2):
                    sl = slice(par * 64, (par + 1) * 64)
                    nc.tensor.matmul(
                        mm_bank[sl, t * C:(t + 1) * C], at2[t][sl, :],
                        ao[sl.start:sl.stop, t * C:(t + 1) * C],
                        start=(t == 0 and par == 0), stop=True,
                        skip_group_check=True)

            mm_sb = consts.tile([P, 2 * C], f32r, name="mm_sb", tag="mm_sb")
            weffb = midB.tile([P, 2 * C], f32, name="weffb", tag="mB")
            weff_sb = consts.tile([P, 2 * C], f32r, name="weff_sb", tag="weff_sb")
            for uu in range(CT):
                nc.vector.tensor_copy(mm_sb[:, uu * C:(uu + 1) * C],
                                      mm_bank[:, uu * C:(uu + 1) * C])
                for t in range(CT):
                    nc.tensor.matmul(
                        weffb[:, t * C:(t + 1) * C],
                        wv_f[uu][:, t * P:(t + 1) * P].bitcast(f32r),
                        mm_sb[:, uu * C:(uu + 1) * C],
                        start=(uu == 0), stop=(uu == CT - 1),
                        skip_group_check=True)
            for t in range(CT):
                nc.vector.tensor_copy(weff_sb[:, t * C:(t + 1) * C],
                                      weffb[:, t * C:(t + 1) * C])
            # last tile-1 pair-copy on Act
            nc.scalar.copy(
                x2t[:, 1, 6 * P:8 * P].bitcast(f32),
                tp1_tiles[3].rearrange("p a q -> p (a q)").bitcast(f32))

            midB_cm.__exit__(None, None, None)
            midA_cm.__exit__(None, None, None)

            # ---- phase 2: out = x2 @ W_eff + bo ----
            p2_cm = tc.tile_pool(name="p2", bufs=2, space="PSUM")
            p2pool = p2_cm.__enter__()
            ostr_cm = tc.tile_pool(name="ostrp", bufs=2)
            ostrpool = ostr_cm.__enter__()

            tpl = None
            p2t = None
            og = None
            for i in range(NCH):
                j = i + LOOK
                if j < NCH:
                    if j % 2 == 0:
                        tpl = work.tile([P, 2, P], f32r, name="tpl",
                                        tag="tp0", bufs=2)
                    nc.tensor.transpose(tpl[:, j % 2, :],
                                        x2s[:, j, P:2 * P], ident_r)
                if i % 2 == 0:
                    p2t = p2pool.tile([P, 2, 2 * C], f32, name="p2t",
                                      tag="p2", bufs=2)
                if i % OB == 0:
                    og = ostrpool.tile([P, OB, C], f32, name="og",
                                       tag="og", bufs=2)
                for t in range(CT):
                    nc.tensor.matmul(
                        p2t[:, i % 2, 0:C],
                        x2t[:, t, i * P:(i + 1) * P],
                        weff_sb[:, t * C:(t + 1) * C],
                        start=(t == 0), stop=(t == CT - 1),
                        skip_group_check=True)
                if j < NCH and j % 2 == 1:
                    nc.scalar.copy(
                        x2t[:, 1, (j - 1) * P:(j + 1) * P].bitcast(f32),
                        tpl.rearrange("p a q -> p (a q)").bitcast(f32))
                if i % 2 == 1:
                    nc.vector.tensor_add(
                        og[:, (i % OB) - 1:(i % OB) + 1, :],
                        p2t[:, :, 0:C], bob2)
                if i % OB == OB - 1:
                    b0 = i - OB + 1
                    dst = bass.AP(
                        tensor=out_d.tensor,
                        offset=out_d.offset + b0 * P * C,
                        ap=[[C, P], [P * C, OB], [1, C]])
                    nc.sync.dma_start(dst, og[:, :, :])
            ostr_cm.__exit__(None, None, None)
            p2_cm.__exit__(None, None, None)

    nc.compile()
    return nc


_NC_CACHE = {}


def _get_nc(n_tokens=_N):
    if n_tokens not in _NC_CACHE:
        _NC_CACHE[n_tokens] = build_nc(n_tokens)
    return _NC_CACHE[n_tokens]


def kernel(x1, x2, Wq, Wk, Wv, Wo, bo, temperature):
    _ensure_paths()
    from concourse.bass_utils import run_bass_kernel_spmd

    B = x1.shape[0]
    nc = _get_nc(x1.shape[1])
    in_maps = []
    for b in range(B):
        in_maps.append({
            "x1": np.ascontiguousarray(x1[b], dtype=np.float32),
            "x2": np.ascontiguousarray(x2[b], dtype=np.float32),
            "Wq": np.asarray(Wq, dtype=np.float32),
            "Wk": np.asarray(Wk, dtype=np.float32),
            "Wv": np.asarray(Wv, dtype=np.float32),
            "Wo": np.asarray(Wo, dtype=np.float32),
            "bo": np.asarray(bo, dtype=np.float32),
            "temperature": np.asarray(temperature, dtype=np.float32),
        })
    res = run_bass_kernel_spmd(nc, in_maps, core_ids=list(range(B)))
    return np.stack([res.results[b]["out"] for b in range(B)]).astype(np.float32)


# revision 24
# speedup vs baseline: 1.2177x; 1.0053x over previous
"""Trainium2 Bass kernel for XCA-style cross-covariance attention.

Reference computation (per batch b):
    q = x1 @ Wq.T ; k = x2 @ Wk.T ; v = x2 @ Wv.T          # [N, C]
    per head h (d=64 channels): L2-normalize q,k along tokens,
    attn = softmax_e((qn^T kn) * temp)                      # [d, d]
    x_cross = attn @ v_h ; out = x_cross @ Wo.T + bo

Gram reformulation (token contractions become PSUM-accumulated Grams):
    S11 = x1^T x1, S21 = x2^T x1, S22 = x2^T x2             # [C, C]
    nq2[c] = sum_m Aq[m,c] * (S11 Aq)[m,c]   (Aq = Wq.T)    # ||q_col||^2
    nk2[c] likewise from S22, Ak
    T2raw = S21^T @ (Ak * temp[cols])        # temp folded into Ak early
    t2 = T2raw * (1/nk)[cols]                # column scaling commutes
    G_h = Aq[:,hb]^T t2[:,hb] ; attn_h = softmax(G_h * (1/nq)[rows])
    M[hb,:] = attn_h^T @ Wo.T[hb,:] ; W_eff = Wv.T @ M
    out = x2 @ W_eff + bo

Schedule (per core; data-parallel over batch B=8 -> 8 cores):
  phase 1 (DMA-paced ~47us): wq/wk/bo/temp first (~0.8us), then x1/x2
    interleaved 4-chunk batches on one queue (x1 through a 16-chunk ring,
    x2 fully staged); per chunk 6 Gram matmuls + 1 transpose of the x2
    chunk's first 128-col tile (f32r, 80ns); wv/wo load after the inputs.
  mid (~8us): serial C x C algebra, q/k chains overlapped across
    PE/DVE/Act, remaining early tile-1 transposes interleaved into PE
    wait gaps.
  phase 2 (store-paced ~24us): per chunk 2 matmuls into a 6-bank PSUM
    ring + 1 lookahead transpose; pair moves with bias add on DVE;
    4-chunk batched stores.
"""

import os
import sys

import numpy as np

_B, _N, _C, _H = 8, 8192, 256, 4
_P = 128  # SBUF partitions


def _ensure_paths():
    for p in ("/root/.axon_site/_ro/trn_rl_repo", "/opt/trn_rl_repo",
              "/root/.axon_site", "/root/.axon_site/_ro/pypackages"):
        if os.path.isdir(p) and p not in sys.path:
            sys.path.append(p)


def build_nc(n_tokens=_N):
    """Build the single-core Bass program (same program SPMD on 8 cores)."""
    _ensure_paths()
    import concourse.bass as bass
    import concourse.mybir as mybir
    import concourse.tile as tile
    from concourse import bacc
    from concourse.masks import make_identity

    f32 = mybir.dt.float32
    f32r = mybir.dt.float32r

    N, C, H = n_tokens, _C, _H
    P = _P
    NCH = N // P          # token chunks of 128
    CT = C // P           # channel tiles (2)
    RING = 16             # x1 ring depth in chunks
    OB = 4                # chunks per output store batch
    PB = 4                # phase-2 PSUM chunk ring depth
    LOOK = 8              # phase-2 transpose lookahead (chunks)

    # load batch sizes (4-chunk batches; soften the tail for earlier
    # last-chunk availability)
    sizes = []
    left = NCH
    while left > 8:
        sizes.append(4)
        left -= 4
    while left > 0:
        sizes.append(2)
        left -= 2

    nc = bacc.Bacc("TRN2", target_bir_lowering=False, debug=False)

    x1_d = nc.dram_tensor("x1", [N, C], f32, kind="ExternalInput").ap()
    x2_d = nc.dram_tensor("x2", [N, C], f32, kind="ExternalInput").ap()
    wq_d = nc.dram_tensor("Wq", [C, C], f32, kind="ExternalInput").ap()
    wk_d = nc.dram_tensor("Wk", [C, C], f32, kind="ExternalInput").ap()
    wv_d = nc.dram_tensor("Wv", [C, C], f32, kind="ExternalInput").ap()
    wo_d = nc.dram_tensor("Wo", [C, C], f32, kind="ExternalInput").ap()
    bo_d = nc.dram_tensor("bo", [C], f32, kind="ExternalInput").ap()
    tp_d = nc.dram_tensor("temperature", [H, 1, 1], f32, kind="ExternalInput").ap()
    out_d = nc.dram_tensor("out", [N, C], f32, kind="ExternalOutput").ap()

    with tile.TileContext(nc) as tc:
        with tc.tile_pool(name="consts", bufs=1) as consts, \
             tc.tile_pool(name="work", bufs=1, space="PSUM") as work:

            ident = consts.tile([P, P], f32, name="ident", tag="ident")
            make_identity(nc, ident)
            ones_f = consts.tile([P, P + 1], f32, name="ones_f", tag="ones_f")
            nc.vector.memset(ones_f, 1.0)
            ones_col = consts.tile([1, P], f32r, name="ones_col", tag="ones_col")
            nc.vector.tensor_copy(ones_col, ones_f[0:1, 0:P])
            ones_red = consts.tile([P, 1], f32r, name="ones_red", tag="ones_red")
            nc.vector.tensor_copy(ones_red, ones_f[:, 0:1])
            # preload the ln/exp/copy activation-table set so the mid phase
            # never pays a 1.3us table switch on the critical chain
            actwarm = consts.tile([1, 1], f32, name="actwarm", tag="actwarm")
            nc.scalar.activation(actwarm, ones_f[0:1, 0:1],
                                 mybir.ActivationFunctionType.Ln)

            # ---- minimal weights first: Wq, Wk, bo, temperature ----
            wq_n = [consts.tile([P, C], f32, name=f"wq_n{t}", tag=f"wq_n{t}") for t in range(CT)]
            wk_n = [consts.tile([P, C], f32, name=f"wk_n{t}", tag=f"wk_n{t}") for t in range(CT)]
            for t in range(CT):
                nc.sync.dma_start(wq_n[t], wq_d[t * P:(t + 1) * P, :])
            for t in range(CT):
                nc.sync.dma_start(wk_n[t], wk_d[t * P:(t + 1) * P, :])
            bo_f = consts.tile([1, C], f32, name="bo_f", tag="bo_f")
            nc.sync.dma_start(bo_f, bo_d.partition_broadcast(1))
            tempsb = consts.tile([1, H], f32, name="tempsb", tag="tempsb")
            nc.sync.dma_start(tempsb, bass.AP(
                tensor=tp_d.tensor, offset=tp_d.offset, ap=[[0, 1], [1, H]]))

            # ---- input staging ----
            # x2 fully resident (no reuse -> no WAR); x1 streams through a
            # 4-batch ring of per-batch pool tiles (WAR at slot granularity).
            x2s = consts.tile([P, NCH, C], f32r, name="x2s", tag="x2s")
            x1pool_cm = tc.tile_pool(name="x1p", bufs=6)
            x1pool = x1pool_cm.__enter__()

            bpos = []
            pos = 0
            for s in sizes:
                bpos.append(pos)
                pos += s
            NB = len(sizes)
            chunk_batch = {}
            for k in range(NB):
                for i in range(bpos[k], bpos[k] + sizes[k]):
                    chunk_batch[i] = k
            # emit batch k's loads at end of chunk last(k-6) (ring WAR pred)
            emit_at = {}
            for k in range(6, NB):
                emit_at.setdefault(bpos[k - 6] + sizes[k - 6] - 1, []).append(k)

            x1tiles = {}

            def load_pair(k):
                s = sizes[k]
                p0 = bpos[k]
                xt = x1pool.tile([P, 4, C], f32r, name=f"x1b{k}", tag="x1b",
                                 bufs=6)
                x1tiles[k] = xt
                src1 = bass.AP(tensor=x1_d.tensor,
                               offset=x1_d.offset + p0 * P * C,
                               ap=[[C, P], [P * C, s], [1, C]]).bitcast(f32r)
                nc.sync.dma_start(xt[:, 0:s, :], src1)
                src2 = bass.AP(tensor=x2_d.tensor,
                               offset=x2_d.offset + p0 * P * C,
                               ap=[[C, P], [P * C, s], [1, C]]).bitcast(f32r)
                nc.sync.dma_start(x2s[:, p0:p0 + s, :], src2)

            for k in range(6):
                load_pair(k)

            # wv/wo load after the phase-1 inputs (emitted later; they are
            # only needed by the mid phase)
            wv_f = [consts.tile([P, C], f32, name=f"wv_f{t}", tag=f"wv_f{t}") for t in range(CT)]
            wo_n = [consts.tile([P, C], f32, name=f"wo_n{t}", tag=f"wo_n{t}") for t in range(CT)]

            # temperature -> flat per-channel row [1, C], then [P, 2C]
            # broadcast (rank-1) -> SBUF for folding into Ak
            tempflat = consts.tile([1, H, C // H], f32, name="tempflat", tag="tempflat")
            for h in range(H):
                nc.vector.tensor_scalar_mul(
                    tempflat[0:1, h, :], ones_f[0:1, 0:C // H],
                    tempsb[0:1, h:h + 1])
            tbrd_p = work.tile([P, 2 * C], f32, name="tbrd", tag="tp", bufs=2)
            tflat = tempflat.rearrange("a h j -> a (h j)")
            for t in range(CT):
                nc.tensor.matmul(tbrd_p[:, t * C:(t + 1) * C], ones_col,
                                 tflat.bitcast(f32r), start=(t == 0),
                                 stop=(t == CT - 1), skip_group_check=True)
            tempbrd = consts.tile([P, 2 * C], f32, name="tempbrd", tag="tempbrd")
            nc.vector.tensor_copy(tempbrd, tbrd_p)

            # bias broadcast rows [P, 2, C] for the phase-2 pair moves
            bob_p = work.tile([P, 2 * C], f32, name="bob", tag="tp", bufs=2)
            for t in range(2):
                nc.tensor.matmul(bob_p[:, t * C:(t + 1) * C], ones_col,
                                 bo_f.bitcast(f32r), start=(t == 0),
                                 stop=(t == 1), skip_group_check=True)
            bob2 = consts.tile([P, 2, C], f32, name="bob2", tag="bob2")
            nc.vector.tensor_copy(bob2.rearrange("p a c -> p (a c)"), bob_p)

            # transposed weights Aq=Wq.T, Ak=Wk.T (early; during DMA fill).
            # Dummy ident-transpose absorbs the identity (Pool) wait; tiny
            # absorber transposes attach each weight-DMA wait to a cheap PE
            # instruction so the real transposes need at most one wait.
            dummy = work.tile([P, P], f32, name="dummy", tag="tp", bufs=2)
            nc.tensor.transpose(dummy, ident, ident)
            for nat in (wq_n, wk_n):
                for tj in range(CT):
                    nc.tensor.transpose(dummy[0:32, :], nat[tj][:, 0:32], ident)

            aq = consts.tile([P, 2 * C], f32r, name="aq", tag="aq")
            ak_raw = consts.tile([P, 2 * C], f32r, name="ak_raw", tag="ak_raw")
            ak_tmp = consts.tile([P, 2 * C], f32r, name="ak_tmp", tag="ak_tmp")
            for ti in range(CT):
                tp = work.tile([P, C], f32, name="tp", tag="tp", bufs=2)
                for tj in range(CT):
                    nc.tensor.transpose(
                        tp[:, tj * P:(tj + 1) * P],
                        wq_n[tj][:, ti * P:(ti + 1) * P], ident)
                nc.vector.tensor_copy(aq[:, ti * C:(ti + 1) * C], tp)
            for ti in range(CT):
                tp = work.tile([P, C], f32, name="tp", tag="tp", bufs=2)
                for tj in range(CT):
                    nc.tensor.transpose(
                        tp[:, tj * P:(tj + 1) * P],
                        wk_n[tj][:, ti * P:(ti + 1) * P], ident)
                nc.vector.tensor_copy(ak_raw[:, ti * C:(ti + 1) * C], tp)
                nc.vector.tensor_mul(ak_tmp[:, ti * C:(ti + 1) * C], tp,
                                     tempbrd[:, ti * C:(ti + 1) * C])

            # ---- phase 1: Gram accumulation + tile-0 x2 transposes ----
            x2t = consts.tile([P, CT, N], f32r, name="x2t", tag="x2t")
            gram_cm = tc.tile_pool(name="gram", bufs=1, space="PSUM")
            gram = gram_cm.__enter__()
            s11p = gram.tile([P, 2 * C], f32, name="s11", tag="s11")
            s21p = gram.tile([P, 2 * C], f32, name="s21", tag="s21")
            s22p = gram.tile([P, 2 * C], f32, name="s22", tag="s22")

            ident_r = ident[:, :].bitcast(f32r)
            tp0 = None
            for i in range(NCH):
                kb = chunk_batch[i]
                x1c = x1tiles[kb][:, i - bpos[kb], :]
                x2c = x2s[:, i, :]
                sp = (i == NCH - 1)
                for t in range(CT):
                    st = (i == 0) and (t == 0)
                    nc.tensor.matmul(
                        s11p[:, t * C:(t + 1) * C], x1c[:, t * P:(t + 1) * P],
                        x1c, start=st, stop=sp, skip_group_check=True)
                for t in range(CT):
                    st = (i == 0) and (t == 0)
                    nc.tensor.matmul(
                        s22p[:, t * C:(t + 1) * C], x2c[:, t * P:(t + 1) * P],
                        x2c, start=st, stop=sp, skip_group_check=True)
                for t in range(CT):
                    st = (i == 0) and (t == 0)
                    nc.tensor.matmul(
                        s21p[:, t * C:(t + 1) * C], x2c[:, t * P:(t + 1) * P],
                        x1c, start=st, stop=sp, skip_group_check=True)
                # tile-0 transpose of this chunk; pair-copy after odd chunks
                if i % 2 == 0:
                    tp0 = work.tile([P, 2, P], f32r, name="tp0", tag="tp0", bufs=2)
                nc.tensor.transpose(tp0[:, i % 2, :], x2c[:, 0:P], ident_r)
                if i % 2 == 1:
                    nc.vector.tensor_copy(
                        x2t[:, 0, (i - 1) * P:(i + 1) * P],
                        tp0.rearrange("p a q -> p (a q)"))
                for k in emit_at.get(i, []):
                    load_pair(k)
            for t in range(CT):
                nc.sync.dma_start(wv_f[t], wv_d[t * P:(t + 1) * P, :])
            for t in range(CT):
                nc.sync.dma_start(wo_n[t], wo_d[t * P:(t + 1) * P, :])
            x1pool_cm.__exit__(None, None, None)

            # ---- post-gram weight processing (PE idle, DVE/Act free) ----
            ao = consts.tile([P, 2 * C], f32r, name="ao", tag="ao")
            for tj in range(CT):
                nc.tensor.transpose(dummy[0:32, :], wo_n[tj][:, 0:32], ident)
            for ti in range(CT):
                tp = work.tile([P, C], f32, name="tp", tag="tp", bufs=2)
                for tj in range(CT):
                    nc.tensor.transpose(
                        tp[:, tj * P:(tj + 1) * P],
                        wo_n[tj][:, ti * P:(ti + 1) * P], ident)
                nc.vector.tensor_copy(ao[:, ti * C:(ti + 1) * C], tp)

            # ---- mid phase ----
            # Gram copies: s22/s21 on DVE (k/T2 chains), s11 on Act.
            s22_sb = consts.tile([P, 2 * C], f32r, name="s22_sb", tag="s22_sb")
            nc.vector.tensor_copy(s22_sb, s22p)
            s21_sb = consts.tile([P, 2 * C], f32r, name="s21_sb", tag="s21_sb")
            nc.vector.tensor_copy(s21_sb, s21p)
            s11_sb = consts.tile([P, 2 * C], f32, name="s11_sb", tag="s11_sb")
            nc.scalar.copy(s11_sb, s11p)
            gram_cm.__exit__(None, None, None)

            midA_cm = tc.tile_pool(name="midA", bufs=1, space="PSUM")
            midA = midA_cm.__enter__()
            midB_cm = tc.tile_pool(name="midB", bufs=3, space="PSUM")
            midB = midB_cm.__enter__()

            # tile-1 transposes for chunks 0..LOOK-1, interleaved into PE
            # wait gaps below; Act pair-copies into x2t tile 1.
            tp1_tiles = []

            def t1_transpose(i):
                if i % 2 == 0:
                    tp1_tiles.append(work.tile([P, 2, P], f32r, name="tp1",
                                               tag="tp0", bufs=2))
                nc.tensor.transpose(tp1_tiles[-1][:, i % 2, :],
                                    x2s[:, i, P:2 * P], ident_r)

            # T2raw = S12 @ (Ak*temp)   [lhsT = s21 tiles]
            t2raw = midA.tile([P, 2 * C], f32, name="t2raw", tag="mA")
            for t in range(CT):
                for uu in range(CT):
                    nc.tensor.matmul(
                        t2raw[:, t * C:(t + 1) * C],
                        s21_sb[:, uu * C + t * P:uu * C + (t + 1) * P],
                        ak_tmp[:, uu * C:(uu + 1) * C],
                        start=(t == 0 and uu == 0), stop=(t == CT - 1 and uu == CT - 1),
                        skip_group_check=True)
            t1_transpose(0)
            t1_transpose(1)
            nc.scalar.copy(
                x2t[:, 1, 0:2 * P].bitcast(f32),
                tp1_tiles[0].rearrange("p a q -> p (a q)").bitcast(f32))

            # u_k = S22 @ Ak ; u_q = S11 @ Aq  (norm chains)
            u_k = midB.tile([P, 2 * C], f32, name="u_k", tag="mB")
            for t in range(CT):
                for uu in range(CT):
                    nc.tensor.matmul(
                        u_k[:, t * C:(t + 1) * C],
                        s22_sb[:, uu * C + t * P:uu * C + (t + 1) * P],
                        ak_raw[:, uu * C:(uu + 1) * C],
                        start=(t == 0 and uu == 0), stop=(t == CT - 1 and uu == CT - 1),
                        skip_group_check=True)
            t1_transpose(2)
            t1_transpose(3)
            nc.scalar.copy(
                x2t[:, 1, 2 * P:4 * P].bitcast(f32),
                tp1_tiles[1].rearrange("p a q -> p (a q)").bitcast(f32))
            u_q = midB.tile([P, 2 * C], f32, name="u_q", tag="mB")
            for t in range(CT):
                for uu in range(CT):
                    nc.tensor.matmul(
                        u_q[:, t * C:(t + 1) * C],
                        s11_sb[:, uu * C + t * P:uu * C + (t + 1) * P].bitcast(f32r),
                        aq[:, uu * C:(uu + 1) * C],
                        start=(t == 0 and uu == 0), stop=(t == CT - 1 and uu == CT - 1),
                        skip_group_check=True)

            vv_k = consts.tile([P, 2 * C], f32r, name="vv_k", tag="vv_k")
            nc.vector.tensor_mul(vv_k, ak_raw, u_k)
            vv_q = consts.tile([P, 2 * C], f32r, name="vv_q", tag="vv_q")
            nc.vector.tensor_mul(vv_q, aq, u_q)

            # nf bank: [1, 2C]: k at cols 0:C, q at cols C:2C
            nf = midB.tile([1, 2 * C], f32, name="nf", tag="mB")
            for t in range(CT):
                nc.tensor.matmul(nf[0:1, 0:C], ones_red,
                                 vv_k[:, t * C:(t + 1) * C],
                                 start=(t == 0), stop=(t == CT - 1),
                                 skip_group_check=True)
            for t in range(CT):
                nc.tensor.matmul(nf[0:1, C:2 * C], ones_red,
                                 vv_q[:, t * C:(t + 1) * C],
                                 start=False, stop=(t == CT - 1),
                                 skip_group_check=True)
            t1_transpose(4)
            t1_transpose(5)

            # 1/sqrt(x) = exp(-0.5*ln(x)): stays within one activation-table
            # set (ln/exp/copy), avoiding table-switch stalls entirely
            lnf = consts.tile([1, 2 * C], f32, name="lnf", tag="lnf")
            nc.scalar.activation(lnf, nf[0:1, :],
                                 mybir.ActivationFunctionType.Ln)
            nk_inv = consts.tile([1, C], f32, name="nk_inv", tag="nk_inv")
            nc.scalar.activation(nk_inv, lnf[0:1, 0:C],
                                 mybir.ActivationFunctionType.Exp, scale=-0.5)
            nq_inv = consts.tile([1, C], f32, name="nq_inv", tag="nq_inv")
            nc.scalar.activation(nq_inv, lnf[0:1, C:2 * C],
                                 mybir.ActivationFunctionType.Exp, scale=-0.5)

            # bnk2 = broadcast(1/nk) twice along free dim  [P, 2C]
            bnk2 = midB.tile([P, 2 * C], f32, name="bnk2", tag="mB")
            for t in range(CT):
                nc.tensor.matmul(bnk2[:, t * C:(t + 1) * C], ones_col,
                                 nk_inv.bitcast(f32r), start=(t == 0),
                                 stop=(t == CT - 1), skip_group_check=True)
            t1_transpose(6)
            t1_transpose(7)
            bnk_sb = consts.tile([P, 2 * C], f32, name="bnk_sb", tag="bnk_sb")
            nc.scalar.copy(bnk_sb, bnk2)
            nc.scalar.copy(
                x2t[:, 1, 4 * P:6 * P].bitcast(f32),
                tp1_tiles[2].rearrange("p a q -> p (a q)").bitcast(f32))

            # t2 = T2raw * (1/nk)[cols]  (by uu halves for G pipelining)
            t2s = consts.tile([P, 2 * C], f32r, name="t2s", tag="t2s")
            nc.vector.tensor_mul(t2s[:, 0:C], t2raw[:, 0:C], bnk_sb[:, 0:C])

            # G pairs + rowscale into one bank [P, 132]
            gbank = work.tile([P, 132], f32, name="gbank", tag="tp", bufs=2)
            for uu in range(CT):
                if uu == 1:
                    nc.vector.tensor_mul(t2s[:, C:2 * C], t2raw[:, C:2 * C],
                                         bnk_sb[:, C:2 * C])
                for t in range(2):      # head pair (2t, 2t+1)
                    for par in range(2):
                        h = 2 * t + par
                        hb = slice(h * 64, (h + 1) * 64)
                        nc.tensor.matmul(
                            gbank[par * 64:(par + 1) * 64, t * 64:(t + 1) * 64],
                            aq[:, uu * C + h * 64:uu * C + (h + 1) * 64],
                            t2s[:, uu * C + h * 64:uu * C + (h + 1) * 64],
                            start=(uu == 0 and t == 0 and par == 0),
                            stop=(uu == CT - 1), skip_group_check=True)
                if uu == 0:
                    # rowscale columns (after the bank's start-zeroing)
                    for t in range(2):
                        nc.tensor.matmul(
                            gbank[:, 128 + 2 * t:129 + 2 * t],
                            nq_inv[0:1, t * P:(t + 1) * P].bitcast(f32r),
                            ones_col[0:1, 0:1],
                            start=False, stop=True, skip_group_check=True)
            rowscale = consts.tile([P, 4], f32, name="rowscale", tag="rowscale")
            nc.vector.tensor_copy(rowscale, gbank[:, 128:132])

            # softmax per pair + M + W_eff
            mm_bank = midB.tile([P, 2 * C], f32, name="mmb", tag="mB")
            at2 = []
            for t in range(2):
                ex = consts.tile([P, 64], f32, name=f"ex{t}", tag=f"ex{t}")
                sume = consts.tile([P, 1], f32, name=f"se{t}", tag=f"se{t}")
                nc.scalar.activation(
                    ex, gbank[:, t * 64:(t + 1) * 64],
                    mybir.ActivationFunctionType.Exp,
                    scale=rowscale[:, 2 * t:2 * t + 1], accum_out=sume)
                sinv = consts.tile([P, 1], f32, name=f"si{t}", tag=f"si{t}")
                nc.vector.reciprocal(sinv, sume)
                at_t = consts.tile([P, 64], f32r, name=f"at{t}", tag=f"at{t}")
                nc.vector.tensor_scalar_mul(at_t, ex, sinv)
                at2.append(at_t)
                for par in range(2):
                    sl = slice(par * 64, (par + 1) * 64)
                    nc.tensor.matmul(
                        mm_bank[sl, t * C:(t + 1) * C], at2[t][sl, :],
                        ao[sl.start:sl.stop, t * C:(t + 1) * C],
                        start=(t == 0 and par == 0), stop=True,
                        skip_group_check=True)

            mm_sb = consts.tile([P, 2 * C], f32r, name="mm_sb", tag="mm_sb")
            weffb = midB.tile([P, 2 * C], f32, name="weffb", tag="mB")
            weff_sb = consts.tile([P, 2 * C], f32r, name="weff_sb", tag="weff_sb")
            for uu in range(CT):
                nc.vector.tensor_copy(mm_sb[:, uu * C:(uu + 1) * C],
                                      mm_bank[:, uu * C:(uu + 1) * C])
                for t in range(CT):
                    nc.tensor.matmul(
                        weffb[:, t * C:(t + 1) * C],
                        wv_f[uu][:, t * P:(t + 1) * P].bitcast(f32r),
                        mm_sb[:, uu * C:(uu + 1) * C],
                        start=(uu == 0), stop=(uu == CT - 1),
                        skip_group_check=True)
            for t in range(CT):
                nc.vector.tensor_copy(weff_sb[:, t * C:(t + 1) * C],
                                      weffb[:, t * C:(t + 1) * C])
            # last tile-1 pair-copy on Act
            nc.scalar.copy(
                x2t[:, 1, 6 * P:8 * P].bitcast(f32),
                tp1_tiles[3].rearrange("p a q -> p (a q)").bitcast(f32))

            midB_cm.__exit__(None, None, None)
            midA_cm.__exit__(None, None, None)

            # ---- phase 2: out = x2 @ W_eff + bo ----
            p2_cm = tc.tile_pool(name="p2", bufs=2, space="PSUM")
            p2pool = p2_cm.__enter__()
            ostr_cm = tc.tile_pool(name="ostrp", bufs=4)
            ostrpool = ostr_cm.__enter__()

            tpl = None
            p2t = None
            og = None
            for i in range(NCH):
                j = i + LOOK
                if j < NCH:
                    if j % 2 == 0:
                        tpl = work.tile([P, 2, P], f32r, name="tpl",
                                        tag="tp0", bufs=2)
                    nc.tensor.transpose(tpl[:, j % 2, :],
                                        x2s[:, j, P:2 * P], ident_r)
                if i % 2 == 0:
                    p2t = p2pool.tile([P, 2, 2 * C], f32, name="p2t",
                                      tag="p2", bufs=2)
                if i % OB == 0:
                    og = ostrpool.tile([P, OB, C], f32, name="og",
                                       tag="og", bufs=2)
                for t in range(CT):
                    nc.tensor.matmul(
                        p2t[:, i % 2, 0:C],
                        x2t[:, t, i * P:(i + 1) * P],
                        weff_sb[:, t * C:(t + 1) * C],
                        start=(t == 0), stop=(t == CT - 1),
                        skip_group_check=True)
                if j < NCH and j % 2 == 1:
                    nc.scalar.copy(
                        x2t[:, 1, (j - 1) * P:(j + 1) * P].bitcast(f32),
                        tpl.rearrange("p a q -> p (a q)").bitcast(f32))
                if i % 2 == 1:
                    nc.vector.tensor_add(
                        og[:, (i % OB) - 1:(i % OB) + 1, :],
                        p2t[:, :, 0:C], bob2)
                if i % OB == OB - 1:
                    b0 = i - OB + 1
                    dst = bass.AP(
                        tensor=out_d.tensor,
                        offset=out_d.offset + b0 * P * C,
                        ap=[[C, P], [P * C, OB], [1, C]])
                    nc.sync.dma_start(dst, og[:, :, :])
            ostr_cm.__exit__(None, None, None)
            p2_cm.__exit__(None, None, None)

    nc.compile()
    return nc


_NC_CACHE = {}


def _get_nc(n_tokens=_N):
    if n_tokens not in _NC_CACHE:
        _NC_CACHE[n_tokens] = build_nc(n_tokens)
    return _NC_CACHE[n_tokens]


def kernel(x1, x2, Wq, Wk, Wv, Wo, bo, temperature):
    _ensure_paths()
    from concourse.bass_utils import run_bass_kernel_spmd

    B = x1.shape[0]
    nc = _get_nc(x1.shape[1])
    in_maps = []
    for b in range(B):
        in_maps.append({
            "x1": np.ascontiguousarray(x1[b], dtype=np.float32),
            "x2": np.ascontiguousarray(x2[b], dtype=np.float32),
            "Wq": np.asarray(Wq, dtype=np.float32),
            "Wk": np.asarray(Wk, dtype=np.float32),
            "Wv": np.asarray(Wv, dtype=np.float32),
            "Wo": np.asarray(Wo, dtype=np.float32),
            "bo": np.asarray(bo, dtype=np.float32),
            "temperature": np.asarray(temperature, dtype=np.float32),
        })
    res = run_bass_kernel_spmd(nc, in_maps, core_ids=list(range(B)))
    return np.stack([res.results[b]["out"] for b in range(B)]).astype(np.float32)


# revision 25
# speedup vs baseline: 1.4086x; 1.1567x over previous
"""Trainium2 Bass kernel for XCA-style cross-covariance attention.

Reference computation (per batch b):
    q = x1 @ Wq.T ; k = x2 @ Wk.T ; v = x2 @ Wv.T          # [N, C]
    per head h (d=64 channels): L2-normalize q,k along tokens,
    attn = softmax_e((qn^T kn) * temp)                      # [d, d]
    x_cross = attn @ v_h ; out = x_cross @ Wo.T + bo

Gram reformulation (token contractions become PSUM-accumulated Grams):
    S11 = x1^T x1, S21 = x2^T x1, S22 = x2^T x2             # [C, C]
    nq2[c] = sum_m Aq[m,c] * (S11 Aq)[m,c]   (Aq = Wq.T)    # ||q_col||^2
    nk2[c] likewise from S22, Ak
    T2raw = S21^T @ (Ak * temp[cols])        # temp folded into Ak early
    t2 = T2raw * (1/nk)[cols]                # column scaling commutes
    G_h = Aq[:,hb]^T t2[:,hb] ; attn_h = softmax(G_h * (1/nq)[rows])
    M[hb,:] = attn_h^T @ Wo.T[hb,:] ; W_eff = Wv.T @ M
    out = x2 @ W_eff + bo

Schedule (per core; data-parallel over batch B=8 -> 8 cores):
  phase 1 (DMA-paced ~47us): wq/wk/bo/temp first (~0.8us), then x1/x2
    interleaved 4-chunk batches on one queue (x1 through a 16-chunk ring,
    x2 fully staged); per chunk 6 Gram matmuls + 1 transpose of the x2
    chunk's first 128-col tile (f32r, 80ns); wv/wo load after the inputs.
  mid (~8us): serial C x C algebra, q/k chains overlapped across
    PE/DVE/Act, remaining early tile-1 transposes interleaved into PE
    wait gaps.
  phase 2 (store-paced ~24us): per chunk 2 matmuls into a 6-bank PSUM
    ring + 1 lookahead transpose; pair moves with bias add on DVE;
    4-chunk batched stores.
"""

import os
import sys

import numpy as np

_B, _N, _C, _H = 8, 8192, 256, 4
_P = 128  # SBUF partitions


def _ensure_paths():
    for p in ("/root/.axon_site/_ro/trn_rl_repo", "/opt/trn_rl_repo",
              "/root/.axon_site", "/root/.axon_site/_ro/pypackages"):
        if os.path.isdir(p) and p not in sys.path:
            sys.path.append(p)


def build_nc(n_tokens=_N):
    """Build the single-core Bass program (same program SPMD on 8 cores)."""
    _ensure_paths()
    import concourse.bass as bass
    import concourse.mybir as mybir
    import concourse.tile as tile
    from concourse import bacc
    from concourse.masks import make_identity

    f32 = mybir.dt.float32
    f32r = mybir.dt.float32r

    N, C, H = n_tokens, _C, _H
    P = _P
    NCH = N // P          # token chunks of 128
    CT = C // P           # channel tiles (2)
    RING = 16             # x1 ring depth in chunks
    OB = 4                # chunks per output store batch
    PB = 4                # phase-2 PSUM chunk ring depth
    LOOK = 8              # phase-2 transpose lookahead (chunks)

    # load batch sizes (4-chunk batches; soften the tail for earlier
    # last-chunk availability)
    sizes = []
    left = NCH
    while left > 8:
        sizes.append(4)
        left -= 4
    while left > 0:
        sizes.append(2)
        left -= 2

    nc = bacc.Bacc("TRN2", target_bir_lowering=False, debug=False)

    x1_d = nc.dram_tensor("x1", [N, C], f32, kind="ExternalInput").ap()
    x2_d = nc.dram_tensor("x2", [N, C], f32, kind="ExternalInput").ap()
    wq_d = nc.dram_tensor("Wq", [C, C], f32, kind="ExternalInput").ap()
    wk_d = nc.dram_tensor("Wk", [C, C], f32, kind="ExternalInput").ap()
    wv_d = nc.dram_tensor("Wv", [C, C], f32, kind="ExternalInput").ap()
    wo_d = nc.dram_tensor("Wo", [C, C], f32, kind="ExternalInput").ap()
    bo_d = nc.dram_tensor("bo", [C], f32, kind="ExternalInput").ap()
    tp_d = nc.dram_tensor("temperature", [H, 1, 1], f32, kind="ExternalInput").ap()
    out_d = nc.dram_tensor("out", [N, C], f32, kind="ExternalOutput").ap()

    with tile.TileContext(nc) as tc:
        with tc.tile_pool(name="consts", bufs=1) as consts, \
             tc.tile_pool(name="work", bufs=1, space="PSUM") as work:

            ident = consts.tile([P, P], f32, name="ident", tag="ident")
            make_identity(nc, ident)
            ones_f = consts.tile([P, P + 1], f32, name="ones_f", tag="ones_f")
            nc.vector.memset(ones_f, 1.0)
            ones_col = consts.tile([1, P], f32r, name="ones_col", tag="ones_col")
            nc.vector.tensor_copy(ones_col, ones_f[0:1, 0:P])
            ones_red = consts.tile([P, 1], f32r, name="ones_red", tag="ones_red")
            nc.vector.tensor_copy(ones_red, ones_f[:, 0:1])
            # preload the ln/exp/copy activation-table set so the mid phase
            # never pays a 1.3us table switch on the critical chain
            actwarm = consts.tile([1, 1], f32, name="actwarm", tag="actwarm")
            nc.scalar.activation(actwarm, ones_f[0:1, 0:1],
                                 mybir.ActivationFunctionType.Ln)

            # ---- minimal weights first: Wq, Wk, bo, temperature ----
            wq_n = [consts.tile([P, C], f32, name=f"wq_n{t}", tag=f"wq_n{t}") for t in range(CT)]
            wk_n = [consts.tile([P, C], f32, name=f"wk_n{t}", tag=f"wk_n{t}") for t in range(CT)]
            for t in range(CT):
                nc.sync.dma_start(wq_n[t], wq_d[t * P:(t + 1) * P, :])
            for t in range(CT):
                nc.sync.dma_start(wk_n[t], wk_d[t * P:(t + 1) * P, :])
            bo_f = consts.tile([1, C], f32, name="bo_f", tag="bo_f")
            nc.sync.dma_start(bo_f, bo_d.partition_broadcast(1))
            tempsb = consts.tile([1, H], f32, name="tempsb", tag="tempsb")
            nc.sync.dma_start(tempsb, bass.AP(
                tensor=tp_d.tensor, offset=tp_d.offset, ap=[[0, 1], [1, H]]))

            # ---- input staging ----
            # x2 fully resident (no reuse -> no WAR); x1 streams through a
            # 4-batch ring of per-batch pool tiles (WAR at slot granularity).
            x2s = consts.tile([P, NCH, C], f32r, name="x2s", tag="x2s")
            x1pool_cm = tc.tile_pool(name="x1p", bufs=6)
            x1pool = x1pool_cm.__enter__()

            bpos = []
            pos = 0
            for s in sizes:
                bpos.append(pos)
                pos += s
            NB = len(sizes)
            chunk_batch = {}
            for k in range(NB):
                for i in range(bpos[k], bpos[k] + sizes[k]):
                    chunk_batch[i] = k
            # emit batch k's loads at end of chunk last(k-6) (ring WAR pred)
            emit_at = {}
            for k in range(6, NB):
                emit_at.setdefault(bpos[k - 6] + sizes[k - 6] - 1, []).append(k)

            x1tiles = {}

            def load_pair(k):
                s = sizes[k]
                p0 = bpos[k]
                xt = x1pool.tile([P, 4, C], f32r, name=f"x1b{k}", tag="x1b",
                                 bufs=6)
                x1tiles[k] = xt
                src1 = bass.AP(tensor=x1_d.tensor,
                               offset=x1_d.offset + p0 * P * C,
                               ap=[[C, P], [P * C, s], [1, C]]).bitcast(f32r)
                nc.sync.dma_start(xt[:, 0:s, :], src1)
                src2 = bass.AP(tensor=x2_d.tensor,
                               offset=x2_d.offset + p0 * P * C,
                               ap=[[C, P], [P * C, s], [1, C]]).bitcast(f32r)
                nc.sync.dma_start(x2s[:, p0:p0 + s, :], src2)

            for k in range(6):
                load_pair(k)

            # wv/wo load after the phase-1 inputs (emitted later; they are
            # only needed by the mid phase)
            wv_f = [consts.tile([P, C], f32, name=f"wv_f{t}", tag=f"wv_f{t}") for t in range(CT)]
            wo_n = [consts.tile([P, C], f32, name=f"wo_n{t}", tag=f"wo_n{t}") for t in range(CT)]

            # temperature -> flat per-channel row [1, C], then [P, 2C]
            # broadcast (rank-1) -> SBUF for folding into Ak
            tempflat = consts.tile([1, H, C // H], f32, name="tempflat", tag="tempflat")
            for h in range(H):
                nc.vector.tensor_scalar_mul(
                    tempflat[0:1, h, :], ones_f[0:1, 0:C // H],
                    tempsb[0:1, h:h + 1])
            tbrd_p = work.tile([P, 2 * C], f32, name="tbrd", tag="tp", bufs=2)
            tflat = tempflat.rearrange("a h j -> a (h j)")
            for t in range(CT):
                nc.tensor.matmul(tbrd_p[:, t * C:(t + 1) * C], ones_col,
                                 tflat.bitcast(f32r), start=(t == 0),
                                 stop=(t == CT - 1), skip_group_check=True)
            tempbrd = consts.tile([P, 2 * C], f32, name="tempbrd", tag="tempbrd")
            nc.vector.tensor_copy(tempbrd, tbrd_p)

            # bias broadcast rows [P, 2, C] for the phase-2 pair moves
            bob_p = work.tile([P, 2 * C], f32, name="bob", tag="tp", bufs=2)
            for t in range(2):
                nc.tensor.matmul(bob_p[:, t * C:(t + 1) * C], ones_col,
                                 bo_f.bitcast(f32r), start=(t == 0),
                                 stop=(t == 1), skip_group_check=True)
            bob2 = consts.tile([P, 2, C], f32, name="bob2", tag="bob2")
            nc.vector.tensor_copy(bob2.rearrange("p a c -> p (a c)"), bob_p)

            # transposed weights Aq=Wq.T, Ak=Wk.T (early; during DMA fill).
            # Dummy ident-transpose absorbs the identity (Pool) wait; tiny
            # absorber transposes attach each weight-DMA wait to a cheap PE
            # instruction so the real transposes need at most one wait.
            dummy = work.tile([P, P], f32, name="dummy", tag="tp", bufs=2)
            nc.tensor.transpose(dummy, ident, ident)
            for nat in (wq_n, wk_n):
                for tj in range(CT):
                    nc.tensor.transpose(dummy[0:32, :], nat[tj][:, 0:32], ident)

            aq = consts.tile([P, 2 * C], f32r, name="aq", tag="aq")
            ak_raw = consts.tile([P, 2 * C], f32r, name="ak_raw", tag="ak_raw")
            ak_tmp = consts.tile([P, 2 * C], f32r, name="ak_tmp", tag="ak_tmp")
            for ti in range(CT):
                tp = work.tile([P, C], f32, name="tp", tag="tp", bufs=2)
                for tj in range(CT):
                    nc.tensor.transpose(
                        tp[:, tj * P:(tj + 1) * P],
                        wq_n[tj][:, ti * P:(ti + 1) * P], ident)
                nc.vector.tensor_copy(aq[:, ti * C:(ti + 1) * C], tp)
            for ti in range(CT):
                tp = work.tile([P, C], f32, name="tp", tag="tp", bufs=2)
                for tj in range(CT):
                    nc.tensor.transpose(
                        tp[:, tj * P:(tj + 1) * P],
                        wk_n[tj][:, ti * P:(ti + 1) * P], ident)
                nc.vector.tensor_copy(ak_raw[:, ti * C:(ti + 1) * C], tp)
                nc.vector.tensor_mul(ak_tmp[:, ti * C:(ti + 1) * C], tp,
                                     tempbrd[:, ti * C:(ti + 1) * C])

            # ---- phase 1: Gram accumulation + tile-0 x2 transposes ----
            x2t = consts.tile([P, CT, N], f32r, name="x2t", tag="x2t")
            gram_cm = tc.tile_pool(name="gram", bufs=1, space="PSUM")
            gram = gram_cm.__enter__()
            s11p = gram.tile([P, 2 * C], f32, name="s11", tag="s11")
            s21p = gram.tile([P, 2 * C], f32, name="s21", tag="s21")
            s22p = gram.tile([P, 2 * C], f32, name="s22", tag="s22")

            ident_r = ident[:, :].bitcast(f32r)
            tp0 = None
            for i in range(NCH):
                kb = chunk_batch[i]
                x1c = x1tiles[kb][:, i - bpos[kb], :]
                x2c = x2s[:, i, :]
                sp = (i == NCH - 1)
                for t in range(CT):
                    st = (i == 0) and (t == 0)
                    nc.tensor.matmul(
                        s11p[:, t * C:(t + 1) * C], x1c[:, t * P:(t + 1) * P],
                        x1c, start=st, stop=sp, skip_group_check=True)
                for t in range(CT):
                    st = (i == 0) and (t == 0)
                    nc.tensor.matmul(
                        s22p[:, t * C:(t + 1) * C], x2c[:, t * P:(t + 1) * P],
                        x2c, start=st, stop=sp, skip_group_check=True)
                for t in range(CT):
                    st = (i == 0) and (t == 0)
                    nc.tensor.matmul(
                        s21p[:, t * C:(t + 1) * C], x2c[:, t * P:(t + 1) * P],
                        x1c, start=st, stop=sp, skip_group_check=True)
                # tile-0 transpose of this chunk; pair-copy after odd chunks
                if i % 2 == 0:
                    tp0 = work.tile([P, 2, P], f32r, name="tp0", tag="tp0", bufs=2)
                nc.tensor.transpose(tp0[:, i % 2, :], x2c[:, 0:P], ident_r)
                if i % 2 == 1:
                    nc.vector.tensor_copy(
                        x2t[:, 0, (i - 1) * P:(i + 1) * P],
                        tp0.rearrange("p a q -> p (a q)"))
                for k in emit_at.get(i, []):
                    load_pair(k)
            for t in range(CT):
                nc.sync.dma_start(wv_f[t], wv_d[t * P:(t + 1) * P, :])
            for t in range(CT):
                nc.sync.dma_start(wo_n[t], wo_d[t * P:(t + 1) * P, :])
            x1pool_cm.__exit__(None, None, None)

            # ---- post-gram weight processing (PE idle, DVE/Act free) ----
            ao = consts.tile([P, 2 * C], f32r, name="ao", tag="ao")
            for tj in range(CT):
                nc.tensor.transpose(dummy[0:32, :], wo_n[tj][:, 0:32], ident)
            for ti in range(CT):
                tp = work.tile([P, C], f32, name="tp", tag="tp", bufs=2)
                for tj in range(CT):
                    nc.tensor.transpose(
                        tp[:, tj * P:(tj + 1) * P],
                        wo_n[tj][:, ti * P:(ti + 1) * P], ident)
                nc.vector.tensor_copy(ao[:, ti * C:(ti + 1) * C], tp)

            # ---- mid phase ----
            # Gram copies: s22/s21 on DVE (k/T2 chains), s11 on Act.
            s22_sb = consts.tile([P, 2 * C], f32r, name="s22_sb", tag="s22_sb")
            nc.vector.tensor_copy(s22_sb, s22p)
            s21_sb = consts.tile([P, 2 * C], f32r, name="s21_sb", tag="s21_sb")
            nc.vector.tensor_copy(s21_sb, s21p)
            s11_sb = consts.tile([P, 2 * C], f32, name="s11_sb", tag="s11_sb")
            nc.scalar.copy(s11_sb, s11p)
            gram_cm.__exit__(None, None, None)

            midA_cm = tc.tile_pool(name="midA", bufs=1, space="PSUM")
            midA = midA_cm.__enter__()
            midB_cm = tc.tile_pool(name="midB", bufs=3, space="PSUM")
            midB = midB_cm.__enter__()

            # tile-1 transposes for chunks 0..LOOK-1, interleaved into PE
            # wait gaps below; Act pair-copies into x2t tile 1.
            tp1_tiles = []

            def t1_transpose(i):
                if i % 2 == 0:
                    tp1_tiles.append(work.tile([P, 2, P], f32r, name="tp1",
                                               tag="tp0", bufs=2))
                nc.tensor.transpose(tp1_tiles[-1][:, i % 2, :],
                                    x2s[:, i, P:2 * P], ident_r)

            # T2raw = S12 @ (Ak*temp)   [lhsT = s21 tiles]
            t2raw = midA.tile([P, 2 * C], f32, name="t2raw", tag="mA")
            for t in range(CT):
                for uu in range(CT):
                    nc.tensor.matmul(
                        t2raw[:, t * C:(t + 1) * C],
                        s21_sb[:, uu * C + t * P:uu * C + (t + 1) * P],
                        ak_tmp[:, uu * C:(uu + 1) * C],
                        start=(t == 0 and uu == 0), stop=(t == CT - 1 and uu == CT - 1),
                        skip_group_check=True)
            t1_transpose(0)
            t1_transpose(1)
            nc.scalar.copy(
                x2t[:, 1, 0:2 * P].bitcast(f32),
                tp1_tiles[0].rearrange("p a q -> p (a q)").bitcast(f32))

            # u_k = S22 @ Ak ; u_q = S11 @ Aq  (norm chains)
            u_k = midB.tile([P, 2 * C], f32, name="u_k", tag="mB")
            for t in range(CT):
                for uu in range(CT):
                    nc.tensor.matmul(
                        u_k[:, t * C:(t + 1) * C],
                        s22_sb[:, uu * C + t * P:uu * C + (t + 1) * P],
                        ak_raw[:, uu * C:(uu + 1) * C],
                        start=(t == 0 and uu == 0), stop=(t == CT - 1 and uu == CT - 1),
                        skip_group_check=True)
            t1_transpose(2)
            t1_transpose(3)
            nc.scalar.copy(
                x2t[:, 1, 2 * P:4 * P].bitcast(f32),
                tp1_tiles[1].rearrange("p a q -> p (a q)").bitcast(f32))
            u_q = midB.tile([P, 2 * C], f32, name="u_q", tag="mB")
            for t in range(CT):
                for uu in range(CT):
                    nc.tensor.matmul(
                        u_q[:, t * C:(t + 1) * C],
                        s11_sb[:, uu * C + t * P:uu * C + (t + 1) * P].bitcast(f32r),
                        aq[:, uu * C:(uu + 1) * C],
                        start=(t == 0 and uu == 0), stop=(t == CT - 1 and uu == CT - 1),
                        skip_group_check=True)

            vv_k = consts.tile([P, 2 * C], f32r, name="vv_k", tag="vv_k")
            nc.vector.tensor_mul(vv_k, ak_raw, u_k)
            vv_q = consts.tile([P, 2 * C], f32r, name="vv_q", tag="vv_q")
            nc.vector.tensor_mul(vv_q, aq, u_q)

            # nf bank: [1, 2C]: k at cols 0:C, q at cols C:2C
            nf = midB.tile([1, 2 * C], f32, name="nf", tag="mB")
            for t in range(CT):
                nc.tensor.matmul(nf[0:1, 0:C], ones_red,
                                 vv_k[:, t * C:(t + 1) * C],
                                 start=(t == 0), stop=(t == CT - 1),
                                 skip_group_check=True)
            for t in range(CT):
                nc.tensor.matmul(nf[0:1, C:2 * C], ones_red,
                                 vv_q[:, t * C:(t + 1) * C],
                                 start=False, stop=(t == CT - 1),
                                 skip_group_check=True)
            t1_transpose(4)
            t1_transpose(5)

            # 1/sqrt(x) = exp(-0.5*ln(x)): stays within one activation-table
            # set (ln/exp/copy), avoiding table-switch stalls entirely
            lnf = consts.tile([1, 2 * C], f32, name="lnf", tag="lnf")
            nc.scalar.activation(lnf, nf[0:1, :],
                                 mybir.ActivationFunctionType.Ln)
            nk_inv = consts.tile([1, C], f32, name="nk_inv", tag="nk_inv")
            nc.scalar.activation(nk_inv, lnf[0:1, 0:C],
                                 mybir.ActivationFunctionType.Exp, scale=-0.5)
            nq_inv = consts.tile([1, C], f32, name="nq_inv", tag="nq_inv")
            nc.scalar.activation(nq_inv, lnf[0:1, C:2 * C],
                                 mybir.ActivationFunctionType.Exp, scale=-0.5)

            # bnk2 = broadcast(1/nk) twice along free dim  [P, 2C]
            bnk2 = midB.tile([P, 2 * C], f32, name="bnk2", tag="mB")
            for t in range(CT):
                nc.tensor.matmul(bnk2[:, t * C:(t + 1) * C], ones_col,
                                 nk_inv.bitcast(f32r), start=(t == 0),
                                 stop=(t == CT - 1), skip_group_check=True)
            t1_transpose(6)
            t1_transpose(7)
            bnk_sb = consts.tile([P, 2 * C], f32, name="bnk_sb", tag="bnk_sb")
            nc.scalar.copy(bnk_sb, bnk2)
            nc.scalar.copy(
                x2t[:, 1, 4 * P:6 * P].bitcast(f32),
                tp1_tiles[2].rearrange("p a q -> p (a q)").bitcast(f32))

            # t2 = T2raw * (1/nk)[cols]  (by uu halves for G pipelining)
            t2s = consts.tile([P, 2 * C], f32r, name="t2s", tag="t2s")
            nc.vector.tensor_mul(t2s[:, 0:C], t2raw[:, 0:C], bnk_sb[:, 0:C])

            # G pairs + rowscale into one bank [P, 132]
            gbank = work.tile([P, 132], f32, name="gbank", tag="tp", bufs=2)
            for uu in range(CT):
                if uu == 1:
                    nc.vector.tensor_mul(t2s[:, C:2 * C], t2raw[:, C:2 * C],
                                         bnk_sb[:, C:2 * C])
                for t in range(2):      # head pair (2t, 2t+1)
                    for par in range(2):
                        h = 2 * t + par
                        hb = slice(h * 64, (h + 1) * 64)
                        nc.tensor.matmul(
                            gbank[par * 64:(par + 1) * 64, t * 64:(t + 1) * 64],
                            aq[:, uu * C + h * 64:uu * C + (h + 1) * 64],
                            t2s[:, uu * C + h * 64:uu * C + (h + 1) * 64],
                            start=(uu == 0 and t == 0 and par == 0),
                            stop=(uu == CT - 1), skip_group_check=True)
                if uu == 0:
                    # rowscale columns (after the bank's start-zeroing)
                    for t in range(2):
                        nc.tensor.matmul(
                            gbank[:, 128 + 2 * t:129 + 2 * t],
                            nq_inv[0:1, t * P:(t + 1) * P].bitcast(f32r),
                            ones_col[0:1, 0:1],
                            start=False, stop=True, skip_group_check=True)
            rowscale = consts.tile([P, 4], f32, name="rowscale", tag="rowscale")
            nc.vector.tensor_copy(rowscale, gbank[:, 128:132])

            # softmax per pair + M + W_eff
            mm_bank = midB.tile([P, 2 * C], f32, name="mmb", tag="mB")
            at2 = []
            for t in range(2):
                ex = consts.tile([P, 64], f32, name=f"ex{t}", tag=f"ex{t}")
                sume = consts.tile([P, 1], f32, name=f"se{t}", tag=f"se{t}")
                nc.scalar.activation(
                    ex, gbank[:, t * 64:(t + 1) * 64],
                    mybir.ActivationFunctionType.Exp,
                    scale=rowscale[:, 2 * t:2 * t + 1], accum_out=sume)
                sinv = consts.tile([P, 1], f32, name=f"si{t}", tag=f"si{t}")
                nc.vector.reciprocal(sinv, sume)
                at_t = consts.tile([P, 64], f32r, name=f"at{t}", tag=f"at{t}")
                nc.vector.tensor_scalar_mul(at_t, ex, sinv)
                at2.append(at_t)
                for par in range(2):
                    sl = slice(par * 64, (par + 1) * 64)
                    nc.tensor.matmul(
                        mm_bank[sl, t * C:(t + 1) * C], at2[t][sl, :],
                        ao[sl.start:sl.stop, t * C:(t + 1) * C],
                        start=(t == 0 and par == 0), stop=True,
                        skip_group_check=True)

            mm_sb = consts.tile([P, 2 * C], f32r, name="mm_sb", tag="mm_sb")
            weffb = midB.tile([P, 2 * C], f32, name="weffb", tag="mB")
            weff_sb = consts.tile([P, 2 * C], f32r, name="weff_sb", tag="weff_sb")
            for uu in range(CT):
                nc.vector.tensor_copy(mm_sb[:, uu * C:(uu + 1) * C],
                                      mm_bank[:, uu * C:(uu + 1) * C])
                for t in range(CT):
                    nc.tensor.matmul(
                        weffb[:, t * C:(t + 1) * C],
                        wv_f[uu][:, t * P:(t + 1) * P].bitcast(f32r),
                        mm_sb[:, uu * C:(uu + 1) * C],
                        start=(uu == 0), stop=(uu == CT - 1),
                        skip_group_check=True)
            for t in range(CT):
                nc.vector.tensor_copy(weff_sb[:, t * C:(t + 1) * C],
                                      weffb[:, t * C:(t + 1) * C])
            # last tile-1 pair-copy on Act
            nc.scalar.copy(
                x2t[:, 1, 6 * P:8 * P].bitcast(f32),
                tp1_tiles[3].rearrange("p a q -> p (a q)").bitcast(f32))

            midB_cm.__exit__(None, None, None)
            midA_cm.__exit__(None, None, None)

            # ---- phase 2: out = x2 @ W_eff + bo ----
            p2_cm = tc.tile_pool(name="p2", bufs=2, space="PSUM")
            p2pool = p2_cm.__enter__()
            ostr_cm = tc.tile_pool(name="ostrp", bufs=4)
            ostrpool = ostr_cm.__enter__()

            tpl = None
            p2t = None
            og = None
            for i in range(NCH):
                j = i + LOOK
                if j < NCH:
                    if j % 2 == 0:
                        tpl = work.tile([P, 2, P], f32r, name="tpl",
                                        tag="tp0", bufs=2)
                    nc.tensor.transpose(tpl[:, j % 2, :],
                                        x2s[:, j, P:2 * P], ident_r)
                if i % 2 == 0:
                    p2t = p2pool.tile([P, 2, 2 * C], f32, name="p2t",
                                      tag="p2", bufs=2)
                if i % OB == 0:
                    og = ostrpool.tile([P, OB, C], f32, name="og",
                                       tag="og", bufs=4)
                for t in range(CT):
                    nc.tensor.matmul(
                        p2t[:, i % 2, 0:C],
                        x2t[:, t, i * P:(i + 1) * P],
                        weff_sb[:, t * C:(t + 1) * C],
                        start=(t == 0), stop=(t == CT - 1),
                        skip_group_check=True)
                if j < NCH and j % 2 == 1:
                    nc.scalar.copy(
                        x2t[:, 1, (j - 1) * P:(j + 1) * P].bitcast(f32),
                        tpl.rearrange("p a q -> p (a q)").bitcast(f32))
                if i % 2 == 1:
                    nc.vector.tensor_add(
                        og[:, (i % OB) - 1:(i % OB) + 1, :],
                        p2t[:, :, 0:C], bob2)
                if i % OB == OB - 1:
                    b0 = i - OB + 1
                    dst = bass.AP(
                        tensor=out_d.tensor,
                        offset=out_d.offset + b0 * P * C,
                        ap=[[C, P], [P * C, OB], [1, C]])
                    nc.sync.dma_start(dst, og[:, :, :])
            ostr_cm.__exit__(None, None, None)
            p2_cm.__exit__(None, None, None)

    nc.compile()
    return nc


_NC_CACHE = {}


def _get_nc(n_tokens=_N):
    if n_tokens not in _NC_CACHE:
        _NC_CACHE[n_tokens] = build_nc(n_tokens)
    return _NC_CACHE[n_tokens]


def kernel(x1, x2, Wq, Wk, Wv, Wo, bo, temperature):
    _ensure_paths()
    from concourse.bass_utils import run_bass_kernel_spmd

    B = x1.shape[0]
    nc = _get_nc(x1.shape[1])
    in_maps = []
    for b in range(B):
        in_maps.append({
            "x1": np.ascontiguousarray(x1[b], dtype=np.float32),
            "x2": np.ascontiguousarray(x2[b], dtype=np.float32),
            "Wq": np.asarray(Wq, dtype=np.float32),
            "Wk": np.asarray(Wk, dtype=np.float32),
            "Wv": np.asarray(Wv, dtype=np.float32),
            "Wo": np.asarray(Wo, dtype=np.float32),
            "bo": np.asarray(bo, dtype=np.float32),
            "temperature": np.asarray(temperature, dtype=np.float32),
        })
    res = run_bass_kernel_spmd(nc, in_maps, core_ids=list(range(B)))
    return np.stack([res.results[b]["out"] for b in range(B)]).astype(np.float32)
